# revision 7
# baseline (speedup 1.0000x reference)
"""Trainium2 Bass kernel for nn_CrossAttention (B=4, C=128, S=32, H=128, W=512).

Strategy (8 NeuronCores, SPMD single program):
  core c -> (batch b=c//2, W-half j=c%2).
  Each core: conv_block K/V over its 256-col half of y (streamed in 8
  W-tiles of 32, fused: K/V never touch DRAM), conv_block Q over x
  (duplicated per pair), attention computed per K/V column for the
  queries whose u falls in that column (host assigns queries to
  per-column slots; Q gathered into slot order on device via one-hot
  matmul). Output: per-slot attention vectors; host scatters to the
  query grid. A second tiny kernel applies the final 3x3 proj conv+relu
  (core c -> (batch, 16-row half)).

All heavy matmuls run as float32r (12-bit mantissa, 1 cyc/row at
free-dim>=256) when USE_F32R, else float32 (exact, 4 cyc/row).
"""
import os
import numpy as np

import concourse.tile as tile
from concourse import bacc, mybir
from concourse.bass_utils import run_bass_kernel_spmd
from concourse.masks import make_identity

# ---- problem constants (hardcoded per spec) ----
BB, CC, S = 4, 128, 32
SS = S * S            # 1024 queries/batch
HH, WW = 128, 512
SCALE = float(32 ** -0.5)
TW = 32               # W-tile width
M = 32                # slots per column
WH = 256              # per-core W half
NSLOT = WH * M        # 8192
NTILE = WH // TW      # 8
NCORE = 8

f32 = mybir.dt.float32
f32r = mybir.dt.float32r
AF = mybir.ActivationFunctionType
ALU = mybir.AluOpType
AX = mybir.AxisListType

USE_F32R = True
COLTILE = False       # col-tiled M=32 matmuls with tile_position (if supported)

TRACE = os.environ.get("KERNEL_TRACE", "0") == "1"
LAST_EXEC_NS = None
LAST_TRACES = []


def _ensure_ntff_hook():
    """The container boot can't register the NTFF profile hook because the
    image's antenv lacks axon_hooks; inject an equivalent module so
    run_bass_kernel_spmd(trace=True) works. Trace-path only."""
    import sys
    import types
    try:
        from antenv.axon_hooks import get_axon_ntff_profile_hook  # noqa
        return
    except ImportError:
        pass
    from trn_agent_boot.trn_boot import _ntff_profile_via_ctypes
    hook = _ntff_profile_via_ctypes('/opt/axon/libaxon_pjrt.so')
    mod = types.ModuleType("antenv.axon_hooks")
    mod.get_axon_ntff_profile_hook = lambda: hook
    mod.set_axon_ntff_profile_hook = lambda h: None
    sys.modules["antenv.axon_hooks"] = mod

_NC_CACHE = {}

WNAMES = ["wq1", "wq2", "wk1", "wk2", "wv1", "wv2"]


def _mdt():
    return f32r if USE_F32R else f32


def _conv_groups(rows, r_grp):
    """Split `rows` into groups of r_grp (last may be smaller)."""
    out = []
    r0 = 0
    while r0 < rows:
        out.append((r0, min(r_grp, rows - r0)))
        r0 += r_grp
    return out


def _chunks(lst, n):
    for i in range(0, len(lst), n):
        yield lst[i:i + n]


def build_kernel_a(reps=1):
    mdt = _mdt()
    nc = bacc.Bacc()

    y_tiles = nc.dram_tensor("y_tiles", [NTILE, 128, 130, 36], mdt,
                             kind="ExternalInput")
    x_slab = nc.dram_tensor("x_slab", [128, 34, 34], mdt, kind="ExternalInput")
    w_dram = {n: nc.dram_tensor(n, [128, 9, 128], mdt, kind="ExternalInput")
              for n in WNAMES}
    bias6 = nc.dram_tensor("bias6", [128, 6], f32, kind="ExternalInput")
    slotq = nc.dram_tensor("slotq", [4, 2048], mdt, kind="ExternalInput")
    iota8 = nc.dram_tensor("iota8", [128, 8], f32, kind="ExternalInput")
    ones1 = nc.dram_tensor("ones1", [1, 128], mdt, kind="ExternalInput")
    edge = nc.dram_tensor("edge", [128, 16], f32, kind="ExternalInput")
    a_out = nc.dram_tensor("a_out", [NSLOT // 128, 128, 128], f32,
                           kind="ExternalOutput")

    from contextlib import ExitStack
    with tile.TileContext(nc) as tc, ExitStack() as ctx:
        wpool = ctx.enter_context(tc.tile_pool(name="weights", bufs=1))
        const = ctx.enter_context(tc.tile_pool(name="const", bufs=1))
        qgp = ctx.enter_context(tc.tile_pool(name="qg", bufs=1))
        ps = ctx.enter_context(tc.tile_pool(name="ps", bufs=8, space="PSUM"))

        def psum_tile():
            return ps.tile([128, 512], f32, tag="ps", name="pst")

        w_sb = {}
        for n in WNAMES:
            t = wpool.tile([128, 9, 128], mdt, tag=n)
            nc.sync.dma_start(out=t[:], in_=w_dram[n][:])
            w_sb[n] = t
        bias_sb = const.tile([128, 6], f32, tag="bias")
        nc.sync.dma_start(out=bias_sb[:], in_=bias6[:])
        iota_sb = const.tile([128, 8], f32, tag="iota")
        nc.sync.dma_start(out=iota_sb[:], in_=iota8[:])
        ones_sb = const.tile([1, 128], mdt, tag="ones")
        nc.sync.dma_start(out=ones_sb[:], in_=ones1[:])
        edge_sb = const.tile([128, 16], f32, tag="edge")
        nc.sync.dma_start(out=edge_sb[:], in_=edge[:])
        ident = const.tile([128, 128], f32, tag="ident")
        make_identity(nc, ident[:])
        zeros34 = const.tile([128, 34], f32, tag="zeros34")
        nc.vector.memset(zeros34[:], 0.0)

        def zfill(ap):
            # zero-fill an f32r view via DVE copy (memset can't emit f32r)
            nc.vector.tensor_copy(out=ap, in_=zeros34[:, :ap.free_size()])

        _build_body(nc, tc, ctx, mdt, psum_tile, w_sb, bias_sb, iota_sb,
                    ones_sb, edge_sb, ident, zfill, qgp,
                    y_tiles, x_slab, slotq, a_out, reps)
    nc.compile()
    return nc


def _build_body(nc, tc, ctx, mdt, psum_tile, w_sb, bias_sb, iota_sb,
                ones_sb, edge_sb, ident, zfill, qgp,
                y_tiles, x_slab, slotq, a_out, reps=1):
    if True:
        Qg = qgp.tile([128, NSLOT], mdt, tag="Qg")

        # ---------------- Q path + slot gather ----------------
        with tc.tile_pool(name="qtmp", bufs=1) as qtmp, \
             tc.tile_pool(name="qt", bufs=1) as qtp, \
             tc.tile_pool(name="msel", bufs=2) as mselp, \
             tc.tile_pool(name="bcast", bufs=2) as bcp:
            xq = qtmp.tile([128, 34, 34], mdt, tag="xq")
            nc.sync.dma_start(out=xq[:], in_=x_slab[:])
            q1 = qtmp.tile([128, 34, 34], mdt, tag="q1")
            q1f = q1[:].rearrange("p a b -> p (a b)")
            zfill(q1f[:, 0:34])                              # row 0
            zfill(q1f[:, 33 * 34:34 * 34])                   # row 33
            zfill(q1[:, :, 0:1].rearrange("p a b -> p (a b)"))    # col 0
            zfill(q1[:, :, 33:34].rearrange("p a b -> p (a b)"))  # col 33
            # conv1-Q: valid 32x32 -> q1[1:33, 1:33]
            for r0 in (0, 16):
                pt = psum_tile()
                for tap in range(9):
                    dy, dx = divmod(tap, 3)
                    nc.tensor.matmul(pt[:, :512], w_sb["wq1"][:, tap, :],
                                     xq[:, r0 + dy:r0 + dy + 16, dx:dx + 32],
                                     start=(tap == 0), stop=(tap == 8))
                # rows 1+r0 .. 1+r0+16 of q1, cols 1..33 (strided out)
                nc.scalar.activation(out=q1[:, 1 + r0:1 + r0 + 16, 1:33],
                                     in_=pt[:, :512].rearrange(
                                         "p (a b) -> p a b", a=16),
                                     func=AF.Relu, bias=bias_sb[:, 0:1],
                                     scale=1.0)
            # conv2-Q -> Q (128, 1024) f32, scaled by SCALE
            q2 = qtmp.tile([128, 1024], f32, tag="q2")
            for r0 in (0, 16):
                pt = psum_tile()
                for tap in range(9):
                    dy, dx = divmod(tap, 3)
                    nc.tensor.matmul(pt[:, :512], w_sb["wq2"][:, tap, :],
                                     q1[:, r0 + dy:r0 + dy + 16, dx:dx + 32],
                                     start=(tap == 0), stop=(tap == 8))
                nc.scalar.activation(out=q2[:, r0 * 32:(r0 + 16) * 32],
                                     in_=pt[:, :512], func=AF.Identity,
                                     bias=bias_sb[:, 1:2], scale=SCALE)
            # transpose Q -> 8 chunks (q, e)
            qt_all = qtp.tile([128, 8, 128], mdt, tag="qt")
            for qc in range(8):
                pt = psum_tile()
                nc.tensor.transpose(pt[:, :128],
                                    q2[:, qc * 128:(qc + 1) * 128], ident[:])
                nc.scalar.copy(out=qt_all[:, qc, :], in_=pt[:, :128])
            # build Qg = Q gathered into slots, via one-hot matmuls
            for blk in range(4):
                sq = bcp.tile([1, 2048], mdt, tag="sq")
                nc.sync.dma_start(out=sq[:], in_=slotq[blk:blk + 1, :])
                bc = bcp.tile([128, 2048], f32, tag="bc")
                for nch in range(4):
                    pb = psum_tile()
                    nc.tensor.matmul(pb[:, :512], ones_sb[:],
                                     sq[:, nch * 512:(nch + 1) * 512],
                                     start=True, stop=True)
                    nc.scalar.copy(out=bc[:, nch * 512:(nch + 1) * 512],
                                   in_=pb[:, :512])
                pq = [psum_tile() for _ in range(4)]
                for qc in range(8):
                    ms = mselp.tile([128, 2048], mdt, tag="ms")
                    nc.vector.tensor_scalar(out=ms[:], in0=bc[:],
                                            scalar1=iota_sb[:, qc:qc + 1],
                                            scalar2=None, op0=ALU.is_equal)
                    for nch in range(4):
                        nc.tensor.matmul(pq[nch][:, :512], qt_all[:, qc, :],
                                         ms[:, nch * 512:(nch + 1) * 512],
                                         start=(qc == 0), stop=(qc == 7))
                for nch in range(4):
                    off = blk * 2048 + nch * 512
                    nc.scalar.copy(out=Qg[:, off:off + 512],
                                   in_=pq[nch][:, :512])

        # ---------------- main loop over W-tiles ----------------
        yp = ctx.enter_context(tc.tile_pool(name="ytile", bufs=2))
        c1p = ctx.enter_context(tc.tile_pool(name="c1", bufs=1))
        k2p = ctx.enter_context(tc.tile_pool(name="k2", bufs=1))
        v2p = ctx.enter_context(tc.tile_pool(name="v2", bufs=1))
        v2tp = ctx.enter_context(tc.tile_pool(name="v2t", bufs=2))
        pp = ctx.enter_context(tc.tile_pool(name="pwork", bufs=2))
        smp = ctx.enter_context(tc.tile_pool(name="smax", bufs=4))

        g1 = _conv_groups(128, 13)   # conv1 valid rows (h 0..127)
        g2 = _conv_groups(128, 16)   # conv2 rows

        for t in list(range(NTILE)) * reps:
            yt = yp.tile([128, 130, 36], mdt, tag="yt")
            nc.sync.dma_start(out=yt[:], in_=y_tiles[t])
            k2 = v2t = None
            for (w1n, w2n, b1i, b2i, kind) in [
                    ("wk1", "wk2", 2, 3, "K"), ("wv1", "wv2", 4, 5, "V")]:
                c1 = c1p.tile([128, 130, 34], mdt, tag="c1")
                c1f = c1[:].rearrange("p a b -> p (a b)")
                zfill(c1f[:, 0:34])
                zfill(c1f[:, 129 * 34:130 * 34])
                for sg in _chunks(g1, 4):
                    pts = [psum_tile() for _ in sg]
                    for tap in range(9):
                        dy, dx = divmod(tap, 3)
                        for (r0, R), pt in zip(sg, pts):
                            nc.tensor.matmul(
                                pt[:, :R * 34], w_sb[w1n][:, tap, :],
                                yt[:, r0 + dy:r0 + dy + R, dx:dx + 34],
                                start=(tap == 0), stop=(tap == 8))
                    for (r0, R), pt in zip(sg, pts):
                        nc.scalar.activation(
                            out=c1f[:, (1 + r0) * 34:(1 + r0 + R) * 34],
                            in_=pt[:, :R * 34], func=AF.Relu,
                            bias=bias_sb[:, b1i:b1i + 1], scale=1.0)
                # zero conv1 halo cols outside the global image (data mask)
                nc.vector.tensor_scalar_mul(c1[:, :, 0:1], c1[:, :, 0:1],
                                            edge_sb[:, 2 * t:2 * t + 1])
                nc.vector.tensor_scalar_mul(c1[:, :, 33:34], c1[:, :, 33:34],
                                            edge_sb[:, 2 * t + 1:2 * t + 2])
                pool2 = k2p if kind == "K" else v2p
                cv2 = pool2.tile([128, 128, 32], mdt, tag="cv2" + kind)
                cv2f = cv2[:].rearrange("p a b -> p (a b)")
                for sg in _chunks(g2, 4):
                    pts = [psum_tile() for _ in sg]
                    for tap in range(9):
                        dy, dx = divmod(tap, 3)
                        for (r0, R), pt in zip(sg, pts):
                            nc.tensor.matmul(
                                pt[:, :R * 32], w_sb[w2n][:, tap, :],
                                c1[:, r0 + dy:r0 + dy + R, dx:dx + 32],
                                start=(tap == 0), stop=(tap == 8))
                    for (r0, R), pt in zip(sg, pts):
                        nc.scalar.activation(
                            out=cv2f[:, r0 * 32:(r0 + R) * 32],
                            in_=pt[:, :R * 32], func=AF.Identity,
                            bias=bias_sb[:, b2i:b2i + 1], scale=1.0)
                if kind == "K":
                    k2 = cv2
                else:
                    # V2T: per-column PE transpose -> (h, w, e)
                    v2t = v2tp.tile([128, 32, 128], mdt, tag="v2t")
                    for wl in range(TW):
                        ptr = psum_tile()
                        nc.tensor.transpose(ptr[:, :128],
                                            cv2[:, :, wl].bitcast(f32),
                                            ident[:])
                        nc.vector.tensor_copy(out=v2t[:, wl, :],
                                              in_=ptr[:, :128])

            # ---------------- attention for this tile ----------------
            for g in range(8):
                p_sb = pp.tile([128, 128], f32, tag="p")
                if COLTILE:
                    sums = smp.tile([128, 4], f32, tag="sums")
                    ps_s = psum_tile()
                    for cg in range(4):
                        wl = g * 4 + cg
                        slot0 = (t * TW + wl) * M
                        nc.tensor.matmul(ps_s[32 * cg:32 * (cg + 1), :128],
                                         Qg[:, slot0:slot0 + 32],
                                         k2[:, :, wl],
                                         start=True, stop=True,
                                         tile_position=(0, 32 * cg))
                    nc.vector.tensor_reduce(out=sums[:, 0:1],
                                            in_=ps_s[:, :128], axis=AX.X,
                                            op=ALU.max, negate=True)
                    nc.scalar.activation(out=p_sb[:], in_=ps_s[:, :128],
                                         func=AF.Exp, bias=sums[:, 0:1],
                                         scale=1.0, accum_out=sums[:, 1:2])
                    nc.vector.reciprocal(out=sums[:, 2:3], in_=sums[:, 1:2])
                    nc.vector.tensor_scalar_mul(p_sb[:], p_sb[:],
                                                sums[:, 2:3])
                else:
                    for cg in range(4):
                        wl = g * 4 + cg
                        slot0 = (t * TW + wl) * M
                        ps_s = psum_tile()
                        nc.tensor.matmul(ps_s[0:32, :128],
                                         Qg[:, slot0:slot0 + 32],
                                         k2[:, :, wl], start=True, stop=True)
                        sm = smp.tile([32, 4], f32, tag="sm")
                        nc.vector.tensor_reduce(out=sm[:, 0:1],
                                                in_=ps_s[0:32, :128],
                                                axis=AX.X, op=ALU.max,
                                                negate=True)
                        nc.scalar.activation(
                            out=p_sb[32 * cg:32 * (cg + 1), :],
                            in_=ps_s[0:32, :128], func=AF.Exp,
                            bias=sm[:, 0:1], scale=1.0,
                            accum_out=sm[:, 1:2])
                        nc.vector.reciprocal(out=sm[:, 2:3], in_=sm[:, 1:2])
                        nc.vector.tensor_scalar_mul(
                            p_sb[32 * cg:32 * (cg + 1), :],
                            p_sb[32 * cg:32 * (cg + 1), :], sm[:, 2:3])
                # transpose p -> (h, slot)
                ptr = psum_tile()
                nc.tensor.transpose(ptr[:, :128], p_sb[:], ident[:])
                pt_sb = pp.tile([128, 128], mdt, tag="ptr")
                nc.scalar.copy(out=pt_sb[:], in_=ptr[:, :128])
                # V contraction
                a_sb = pp.tile([128, 128], f32, tag="a")
                if COLTILE:
                    ps_a = psum_tile()
                    for cg in range(4):
                        wl = g * 4 + cg
                        nc.tensor.matmul(ps_a[32 * cg:32 * (cg + 1), :128],
                                         pt_sb[:, 32 * cg:32 * (cg + 1)],
                                         v2t[:, wl, :], start=True, stop=True,
                                         tile_position=(0, 32 * cg))
                    nc.scalar.copy(out=a_sb[:], in_=ps_a[:, :128])
                else:
                    for cg in range(4):
                        wl = g * 4 + cg
                        ps_a = psum_tile()
                        nc.tensor.matmul(ps_a[0:32, :128],
                                         pt_sb[:, 32 * cg:32 * (cg + 1)],
                                         v2t[:, wl, :], start=True, stop=True)
                        nc.scalar.copy(out=a_sb[32 * cg:32 * (cg + 1), :],
                                       in_=ps_a[0:32, :128])
                nc.sync.dma_start(out=a_out[t * 8 + g], in_=a_sb[:])


def build_kernel_b():
    mdt = _mdt()
    nc = bacc.Bacc()
    a_slab = nc.dram_tensor("a_slab", [128, 18, 34], mdt, kind="ExternalInput")
    wp = nc.dram_tensor("wp", [128, 9, 128], mdt, kind="ExternalInput")
    bp = nc.dram_tensor("bp", [128, 1], f32, kind="ExternalInput")
    z_out = nc.dram_tensor("z_out", [128, 512], f32, kind="ExternalOutput")

    with tile.TileContext(nc) as tc:
        with tc.tile_pool(name="sb", bufs=1) as sb, \
             tc.tile_pool(name="ps", bufs=2, space="PSUM") as ps:
            a_sb = sb.tile([128, 18, 34], mdt)
            nc.sync.dma_start(out=a_sb[:], in_=a_slab[:])
            wp_sb = sb.tile([128, 9, 128], mdt)
            nc.sync.dma_start(out=wp_sb[:], in_=wp[:])
            bp_sb = sb.tile([128, 1], f32)
            nc.sync.dma_start(out=bp_sb[:], in_=bp[:])
            pt = ps.tile([128, 512], f32)
            for tap in range(9):
                dy, dx = divmod(tap, 3)
                nc.tensor.matmul(pt[:], wp_sb[:, tap, :],
                                 a_sb[:, dy:dy + 16, dx:dx + 32],
                                 start=(tap == 0), stop=(tap == 8))
            z_sb = sb.tile([128, 512], f32)
            nc.scalar.activation(out=z_sb[:], in_=pt[:], func=AF.Relu,
                                 bias=bp_sb[:, 0:1], scale=1.0)
            nc.sync.dma_start(out=z_out[:], in_=z_sb[:])
    nc.compile()
    return nc


def _round12(a):
    if not USE_F32R:
        return np.ascontiguousarray(a, np.float32)
    b = np.ascontiguousarray(a, np.float32).view(np.uint32)
    b = (b + np.uint32(0x400)) & np.uint32(0xFFFFF800)
    return b.view(np.float32)


def _get_nc(which):
    key = (which, USE_F32R, COLTILE)
    if key not in _NC_CACHE:
        _NC_CACHE[key] = (build_kernel_a() if which == "a"
                          else build_kernel_b())
    return _NC_CACHE[key]


def _prep_core_a(xr, yr, uc, wt, bias6, b, j):
    """Per-core host prep. xr/yr pre-rounded full arrays."""
    y = yr[b]                      # (128, 128, 512)
    x = xr[b]                      # (128, 32, 32)
    u = uc[b].reshape(SS)          # int64 in [0, 512)

    x_slab = np.zeros((128, 34, 34), np.float32)
    x_slab[:, 1:33, 1:33] = x

    y_slab = np.zeros((128, 130, 260), np.float32)
    lo, hi = WH * j - 2, WH * j + WH + 2
    glo, ghi = max(lo, 0), min(hi, WW)
    y_slab[:, 1:129, (glo - lo):(ghi - lo)] = y[:, :, glo:ghi]
    y_tiles = np.stack([y_slab[:, :, TW * t:TW * t + 36]
                        for t in range(NTILE)])

    local = u - WH * j
    mask = (local >= 0) & (local < WH)
    slotq = np.full((NSLOT,), 2000.0, np.float32)
    counts = np.zeros(WH, np.int64)
    for q in range(SS):
        if mask[q]:
            w = int(local[q])
            r = counts[w]
            assert r < M, f"column {w} overflows {M} slots"
            slotq[w * M + r] = float(q)
            counts[w] += 1

    edge = np.ones((128, 16), np.float32)
    if j == 0:
        edge[:, 0] = 0.0        # tile 0, col0 -> global col -1
    else:
        edge[:, 2 * (NTILE - 1) + 1] = 0.0   # last tile col33 -> global 512

    iota8 = (np.arange(8, dtype=np.float32)[None, :] * 128
             + np.arange(128, dtype=np.float32)[:, None])

    in_map = {
        "y_tiles": y_tiles,
        "x_slab": x_slab,
        "bias6": bias6,
        "slotq": slotq.reshape(4, 2048),
        "iota8": iota8,
        "ones1": np.ones((1, 128), np.float32),
        "edge": edge,
    }
    in_map.update(wt)
    return in_map, slotq


def kernel(x, y, u, q_w1, q_b1, q_w2, q_b2, k_w1, k_b1, k_w2, k_b2,
           v_w1, v_b1, v_w2, v_b2, proj_w, proj_b):
    x = np.asarray(x, np.float32)
    y = np.asarray(y, np.float32)
    u_in = np.asarray(u)
    uc = np.clip(u_in, 0, WW - 1).astype(np.int64)

    xr, yr = _round12(x), _round12(y)
    wsrc = {"wq1": q_w1, "wq2": q_w2, "wk1": k_w1, "wk2": k_w2,
            "wv1": v_w1, "wv2": v_w2}
    wt = {n: _round12(np.asarray(w, np.float32)
                      .transpose(1, 2, 3, 0).reshape(128, 9, 128))
          for n, w in wsrc.items()}
    bias6 = np.stack([
        np.asarray(q_b1, np.float32),
        np.asarray(q_b2, np.float32) * np.float32(SCALE),
        np.asarray(k_b1, np.float32), np.asarray(k_b2, np.float32),
        np.asarray(v_b1, np.float32), np.asarray(v_b2, np.float32),
    ], axis=1)                     # (128, 6)

    in_maps, slot_maps = [], []
    for c in range(NCORE):
        im, sq = _prep_core_a(xr, yr, uc, wt, bias6, c // 2, c % 2)
        in_maps.append(im)
        slot_maps.append(sq)

    global LAST_EXEC_NS, LAST_TRACES
    LAST_TRACES = []
    if TRACE:
        _ensure_ntff_hook()
    tkw = dict(trace=True, trace_cores=list(range(NCORE))) if TRACE else {}
    nc_a = _get_nc("a")
    res_a = run_bass_kernel_spmd(nc_a, in_maps, list(range(NCORE)), **tkw)

    a_full = np.zeros((BB, SS, 128), np.float32)
    for c in range(NCORE):
        flat = res_a.results[c]["a_out"].reshape(NSLOT, 128)
        sq = slot_maps[c]
        valid = sq < 1024
        a_full[c // 2][sq[valid].astype(np.int64)] = flat[valid]
    a_img = a_full.transpose(0, 2, 1).reshape(BB, 128, S, S)

    wpr = _round12(np.asarray(proj_w, np.float32)
                   .transpose(1, 2, 3, 0).reshape(128, 9, 128))
    bpr = np.asarray(proj_b, np.float32).reshape(128, 1)
    in_maps_b = []
    for c in range(NCORE):
        b, rh = c // 2, c % 2
        a_slab = np.zeros((128, 18, 34), np.float32)
        r0 = 16 * rh
        rlo, rhi = max(r0 - 1, 0), min(r0 + 17, S)
        a_slab[:, (rlo - (r0 - 1)):(rhi - (r0 - 1)), 1:33] = \
            _round12(a_img[b, :, rlo:rhi, :])
        in_maps_b.append({"a_slab": a_slab, "wp": wpr, "bp": bpr})

    nc_b = _get_nc("b")
    res_b = run_bass_kernel_spmd(nc_b, in_maps_b, list(range(NCORE)), **tkw)
    if TRACE:
        LAST_EXEC_NS = (res_a.exec_time_ns or 0) + (res_b.exec_time_ns or 0)
        LAST_TRACES = [res_a, res_b]

    z = np.zeros((BB, 128, S, S), np.float32)
    for c in range(NCORE):
        b, rh = c // 2, c % 2
        z[b, :, 16 * rh:16 * rh + 16, :] = \
            res_b.results[c]["z_out"].reshape(128, 16, 32)
    return z



# revision 12
# speedup vs baseline: 1.2882x; 1.2882x over previous
"""Trainium2 Bass kernel for nn_CrossAttention (B=4, C=128, S=32, H=128, W=512).

Strategy (8 NeuronCores, SPMD single program):
  core c -> (batch b=c//2, W-half j=c%2).
  Each core: conv_block K/V over its 256-col half of y (streamed in 8
  W-tiles of 32, fused: K/V never touch DRAM), conv_block Q over x
  (duplicated per pair), attention computed per K/V column for the
  queries whose u falls in that column (host assigns queries to
  per-column slots; Q gathered into slot order on device via one-hot
  matmul). Output: per-slot attention vectors; host scatters to the
  query grid. A second tiny kernel applies the final 3x3 proj conv+relu
  (core c -> (batch, 16-row half)).

All heavy matmuls run as float32r (12-bit mantissa, 1 cyc/row at
free-dim>=256) when USE_F32R, else float32 (exact, 4 cyc/row).
"""
import os
import numpy as np

import concourse.tile as tile
from concourse import bacc, mybir
from concourse.bass_utils import run_bass_kernel_spmd
from concourse.masks import make_identity

# ---- problem constants (hardcoded per spec) ----
BB, CC, S = 4, 128, 32
SS = S * S            # 1024 queries/batch
HH, WW = 128, 512
SCALE = float(32 ** -0.5)
TW = 32               # W-tile width
M = 32                # slots per column
WH = 256              # per-core W half
NSLOT = WH * M        # 8192
NTILE = WH // TW      # 8
NCORE = 8

f32 = mybir.dt.float32
f32r = mybir.dt.float32r
AF = mybir.ActivationFunctionType
ALU = mybir.AluOpType
AX = mybir.AxisListType

USE_F32R = True
COLTILE = False       # col-tiled M=32 matmuls with tile_position (if supported)

TRACE = os.environ.get("KERNEL_TRACE", "0") == "1"
LAST_EXEC_NS = None
LAST_TRACES = []


def _ensure_ntff_hook():
    """The container boot can't register the NTFF profile hook because the
    image's antenv lacks axon_hooks; inject an equivalent module so
    run_bass_kernel_spmd(trace=True) works. Trace-path only."""
    import sys
    import types
    try:
        from antenv.axon_hooks import get_axon_ntff_profile_hook  # noqa
        return
    except ImportError:
        pass
    from trn_agent_boot.trn_boot import _ntff_profile_via_ctypes
    hook = _ntff_profile_via_ctypes('/opt/axon/libaxon_pjrt.so')
    mod = types.ModuleType("antenv.axon_hooks")
    mod.get_axon_ntff_profile_hook = lambda: hook
    mod.set_axon_ntff_profile_hook = lambda h: None
    sys.modules["antenv.axon_hooks"] = mod

_NC_CACHE = {}

WNAMES = ["wq1", "wq2", "wk1", "wk2", "wv1", "wv2"]


def _mdt():
    return f32r if USE_F32R else f32


def _conv_groups(rows, r_grp):
    """Split `rows` into groups of r_grp (last may be smaller)."""
    out = []
    r0 = 0
    while r0 < rows:
        out.append((r0, min(r_grp, rows - r0)))
        r0 += r_grp
    return out


def _chunks(lst, n):
    for i in range(0, len(lst), n):
        yield lst[i:i + n]


def build_kernel_a(reps=1):
    mdt = _mdt()
    nc = bacc.Bacc()

    y_tiles = nc.dram_tensor("y_tiles", [NTILE, 128, 130, 36], mdt,
                             kind="ExternalInput")
    x_slab = nc.dram_tensor("x_slab", [128, 34, 34], mdt, kind="ExternalInput")
    w_dram = {n: nc.dram_tensor(n, [128, 9, 128], mdt, kind="ExternalInput")
              for n in WNAMES}
    bias6 = nc.dram_tensor("bias6", [128, 6], f32, kind="ExternalInput")
    slotq = nc.dram_tensor("slotq", [4, 2048], mdt, kind="ExternalInput")
    iota8 = nc.dram_tensor("iota8", [128, 8], f32, kind="ExternalInput")
    ones1 = nc.dram_tensor("ones1", [1, 128], mdt, kind="ExternalInput")
    edge = nc.dram_tensor("edge", [128, 16], f32, kind="ExternalInput")
    a_out = nc.dram_tensor("a_out", [NSLOT // 128, 128, 128], f32,
                           kind="ExternalOutput")

    from contextlib import ExitStack
    with tile.TileContext(nc) as tc, ExitStack() as ctx:
        wpool = ctx.enter_context(tc.tile_pool(name="weights", bufs=1))
        const = ctx.enter_context(tc.tile_pool(name="const", bufs=1))
        qgp = ctx.enter_context(tc.tile_pool(name="qg", bufs=1))
        ps = ctx.enter_context(tc.tile_pool(name="ps", bufs=8, space="PSUM"))

        def psum_tile():
            return ps.tile([128, 512], f32, tag="ps", name="pst")

        w_sb = {}
        for n in WNAMES:
            t = wpool.tile([128, 9, 128], mdt, tag=n)
            nc.sync.dma_start(out=t[:], in_=w_dram[n][:])
            w_sb[n] = t
        bias_sb = const.tile([128, 6], f32, tag="bias")
        nc.sync.dma_start(out=bias_sb[:], in_=bias6[:])
        iota_sb = const.tile([128, 8], f32, tag="iota")
        nc.sync.dma_start(out=iota_sb[:], in_=iota8[:])
        ones_sb = const.tile([1, 128], mdt, tag="ones")
        nc.sync.dma_start(out=ones_sb[:], in_=ones1[:])
        edge_sb = const.tile([128, 16], f32, tag="edge")
        nc.sync.dma_start(out=edge_sb[:], in_=edge[:])
        ident = const.tile([128, 128], f32, tag="ident")
        make_identity(nc, ident[:])
        zeros34 = const.tile([128, 34], f32, tag="zeros34")
        nc.vector.memset(zeros34[:], 0.0)

        def zfill(ap):
            # zero-fill an f32r view via DVE copy (memset can't emit f32r)
            nc.vector.tensor_copy(out=ap, in_=zeros34[:, :ap.free_size()])

        _build_body(nc, tc, ctx, mdt, psum_tile, w_sb, bias_sb, iota_sb,
                    ones_sb, edge_sb, ident, zfill, qgp,
                    y_tiles, x_slab, slotq, a_out, reps)
    nc.compile()
    return nc


def _build_body(nc, tc, ctx, mdt, psum_tile, w_sb, bias_sb, iota_sb,
                ones_sb, edge_sb, ident, zfill, qgp,
                y_tiles, x_slab, slotq, a_out, reps=1):
    if True:
        Qg = qgp.tile([128, NSLOT], mdt, tag="Qg")

        # y tile pool hoisted above the Q path so the first two y DMAs can
        # overlap Q computation (no SBUF WAR against Q-path temporaries).
        yp = ctx.enter_context(tc.tile_pool(name="ytile", bufs=2))
        tile_order = list(range(NTILE)) * reps
        yt_cur = yp.tile([128, 130, 36], mdt, tag="yt")
        nc.sync.dma_start(out=yt_cur[:], in_=y_tiles[tile_order[0]])

        # ---------------- Q path + slot gather ----------------
        with tc.tile_pool(name="qtmp", bufs=1) as qtmp, \
             tc.tile_pool(name="qt", bufs=1) as qtp, \
             tc.tile_pool(name="msel", bufs=2) as mselp, \
             tc.tile_pool(name="bcast", bufs=2) as bcp:
            xq = qtmp.tile([128, 34, 34], mdt, tag="xq")
            nc.sync.dma_start(out=xq[:], in_=x_slab[:])
            q1 = qtmp.tile([128, 34, 34], mdt, tag="q1")
            q1f = q1[:].rearrange("p a b -> p (a b)")
            zfill(q1f[:, 0:34])                              # row 0
            zfill(q1f[:, 33 * 34:34 * 34])                   # row 33
            zfill(q1[:, :, 0:1].rearrange("p a b -> p (a b)"))    # col 0
            zfill(q1[:, :, 33:34].rearrange("p a b -> p (a b)"))  # col 33
            # conv1-Q: valid 32x32 -> q1[1:33, 1:33]
            for r0 in (0, 16):
                pt = psum_tile()
                for tap in range(9):
                    dy, dx = divmod(tap, 3)
                    nc.tensor.matmul(pt[:, :512], w_sb["wq1"][:, tap, :],
                                     xq[:, r0 + dy:r0 + dy + 16, dx:dx + 32],
                                     start=(tap == 0), stop=(tap == 8))
                # rows 1+r0 .. 1+r0+16 of q1, cols 1..33 (strided out)
                nc.scalar.activation(out=q1[:, 1 + r0:1 + r0 + 16, 1:33],
                                     in_=pt[:, :512].rearrange(
                                         "p (a b) -> p a b", a=16),
                                     func=AF.Relu, bias=bias_sb[:, 0:1],
                                     scale=1.0)
            # conv2-Q -> Q (128, 1024) f32, scaled by SCALE
            q2 = qtmp.tile([128, 1024], f32, tag="q2")
            for r0 in (0, 16):
                pt = psum_tile()
                for tap in range(9):
                    dy, dx = divmod(tap, 3)
                    nc.tensor.matmul(pt[:, :512], w_sb["wq2"][:, tap, :],
                                     q1[:, r0 + dy:r0 + dy + 16, dx:dx + 32],
                                     start=(tap == 0), stop=(tap == 8))
                nc.scalar.activation(out=q2[:, r0 * 32:(r0 + 16) * 32],
                                     in_=pt[:, :512], func=AF.Identity,
                                     bias=bias_sb[:, 1:2], scale=SCALE)
            # transpose Q -> 8 chunks (q, e)
            qt_all = qtp.tile([128, 8, 128], mdt, tag="qt")
            for qc in range(8):
                pt = psum_tile()
                nc.tensor.transpose(pt[:, :128],
                                    q2[:, qc * 128:(qc + 1) * 128], ident[:])
                nc.scalar.copy(out=qt_all[:, qc, :], in_=pt[:, :128])
            # build Qg = Q gathered into slots, via one-hot matmuls
            for blk in range(4):
                sq = bcp.tile([1, 2048], mdt, tag="sq")
                nc.sync.dma_start(out=sq[:], in_=slotq[blk:blk + 1, :])
                bc = bcp.tile([128, 2048], f32, tag="bc")
                for nch in range(4):
                    pb = psum_tile()
                    nc.tensor.matmul(pb[:, :512], ones_sb[:],
                                     sq[:, nch * 512:(nch + 1) * 512],
                                     start=True, stop=True)
                    nc.scalar.copy(out=bc[:, nch * 512:(nch + 1) * 512],
                                   in_=pb[:, :512])
                pq = [psum_tile() for _ in range(4)]
                for qc in range(8):
                    ms = mselp.tile([128, 2048], mdt, tag="ms")
                    nc.vector.tensor_scalar(out=ms[:], in0=bc[:],
                                            scalar1=iota_sb[:, qc:qc + 1],
                                            scalar2=None, op0=ALU.is_equal)
                    for nch in range(4):
                        nc.tensor.matmul(pq[nch][:, :512], qt_all[:, qc, :],
                                         ms[:, nch * 512:(nch + 1) * 512],
                                         start=(qc == 0), stop=(qc == 7))
                for nch in range(4):
                    off = blk * 2048 + nch * 512
                    nc.scalar.copy(out=Qg[:, off:off + 512],
                                   in_=pq[nch][:, :512])

        # ---------------- main loop over W-tiles ----------------
        c1p = ctx.enter_context(tc.tile_pool(name="c1", bufs=1))
        k2p = ctx.enter_context(tc.tile_pool(name="k2", bufs=1))
        v2p = ctx.enter_context(tc.tile_pool(name="v2", bufs=1))
        v2tp = ctx.enter_context(tc.tile_pool(name="v2t", bufs=2))
        pp = ctx.enter_context(tc.tile_pool(name="pwork", bufs=2))
        pbank = ctx.enter_context(tc.tile_pool(name="pbank", bufs=1))
        smp = ctx.enter_context(tc.tile_pool(name="smax", bufs=1))

        g1 = _conv_groups(128, 15)   # conv1 valid rows (h 0..127)
        g2 = _conv_groups(128, 16)   # conv2 rows

        def conv1(w1n, b1i, t, yt):
            c1 = c1p.tile([128, 130, 34], mdt, tag="c1")
            c1f = c1[:].rearrange("p a b -> p (a b)")
            zfill(c1f[:, 0:34])
            zfill(c1f[:, 129 * 34:130 * 34])
            for ci, sg in enumerate(_chunks(g1, 4)):
                pts = [psum_tile() for _ in sg]
                for tap in range(9):
                    dy, dx = divmod(tap, 3)
                    for (r0, R), pt in zip(sg, pts):
                        nc.tensor.matmul(
                            pt[:, :R * 34], w_sb[w1n][:, tap, :],
                            yt[:, r0 + dy:r0 + dy + R, dx:dx + 34],
                            start=(tap == 0), stop=(tap == 8))
                for (r0, R), pt in zip(sg, pts):
                    nc.scalar.activation(
                        out=c1f[:, (1 + r0) * 34:(1 + r0 + R) * 34],
                        in_=pt[:, :R * 34], func=AF.Relu,
                        bias=bias_sb[:, b1i:b1i + 1], scale=1.0)
                yield ci
            # zero conv1 halo cols outside the global image (data mask)
            nc.vector.tensor_scalar_mul(c1[:, :, 0:1], c1[:, :, 0:1],
                                        edge_sb[:, 2 * t:2 * t + 1])
            nc.vector.tensor_scalar_mul(c1[:, :, 33:34], c1[:, :, 33:34],
                                        edge_sb[:, 2 * t + 1:2 * t + 2])
            yield c1

        def conv2(w2n, b2i, c1, pool2, kind):
            # K is stored (e, w, h) so a 4-column slab is one contiguous
            # 512-wide moving operand for the batched QK matmul.
            if kind == "K":
                cv2 = pool2.tile([128, 32, 128], mdt, tag="cv2K")
            else:
                cv2 = pool2.tile([128, 128, 32], mdt, tag="cv2V")
                cv2f = cv2[:].rearrange("p a b -> p (a b)")
            for sg in _chunks(g2, 4):
                pts = [psum_tile() for _ in sg]
                for tap in range(9):
                    dy, dx = divmod(tap, 3)
                    for (r0, R), pt in zip(sg, pts):
                        nc.tensor.matmul(
                            pt[:, :R * 32], w_sb[w2n][:, tap, :],
                            c1[:, r0 + dy:r0 + dy + R, dx:dx + 32],
                            start=(tap == 0), stop=(tap == 8))
                for (r0, R), pt in zip(sg, pts):
                    if kind == "K":
                        out_ap = cv2[:, :, r0:r0 + R].transpose([0, 2, 1])
                        in_ap = pt[:, :R * 32].rearrange(
                            "p (a b) -> p a b", a=R)
                    else:
                        out_ap = cv2f[:, r0 * 32:(r0 + R) * 32]
                        in_ap = pt[:, :R * 32]
                    nc.scalar.activation(
                        out=out_ap, in_=in_ap, func=AF.Identity,
                        bias=bias_sb[:, b2i:b2i + 1], scale=1.0)
            return cv2

        def qk_softmax(t, g, k2):
            # scores for 128 slots (4 columns x 32 slots) in one matmul:
            # out[slot, c*128+h]; then exp of the 4 diagonal blocks.
            slot0 = (t * TW + g * 4) * M
            ps_s = psum_tile()
            nc.tensor.matmul(ps_s[:, :512], Qg[:, slot0:slot0 + 128],
                             k2[:, g * 4:g * 4 + 4, :].rearrange(
                                 "p a b -> p (a b)"),
                             start=True, stop=True)
            p_sb = pbank.tile([128, 128], f32, tag=f"p{g}")
            for cg in range(4):
                nc.scalar.activation(
                    out=p_sb[32 * cg:32 * (cg + 1), :],
                    in_=ps_s[32 * cg:32 * (cg + 1), 128 * cg:128 * (cg + 1)],
                    func=AF.Exp, bias=0.0, scale=1.0)
            sums = smp.tile([128, 2], f32, tag=f"sm{g}")
            nc.vector.tensor_reduce(out=sums[:, 0:1], in_=p_sb[:],
                                    axis=AX.X, op=ALU.add)
            nc.vector.reciprocal(out=sums[:, 1:2], in_=sums[:, 0:1])
            return p_sb, sums

        for ti, t in enumerate(tile_order):
            yt = yt_cur
            if ti + 1 < len(tile_order):
                yt_next = yp.tile([128, 130, 36], mdt, tag="yt")
                nc.sync.dma_start(out=yt_next[:],
                                  in_=y_tiles[tile_order[ti + 1]])
            else:
                yt_next = None

            # K path
            ck = conv1("wk1", 2, t, yt)
            for _ in range(3):
                next(ck)
            c1 = next(ck)
            k2 = conv2("wk2", 3, c1, k2p, "K")

            # QK + softmax interleaved with the V path for PSUM slack
            psm = [None] * 8
            for g in range(4):
                psm[g] = qk_softmax(t, g, k2)
            cv = conv1("wv1", 4, t, yt)
            next(cv)
            for g in range(4, 8):
                psm[g] = qk_softmax(t, g, k2)
            next(cv)
            next(cv)
            c1 = next(cv)
            v2 = conv2("wv2", 5, c1, v2p, "V")

            # V2T: per-column PE transpose -> (h, w, e), 4 columns per bank
            v2t = v2tp.tile([128, 32, 128], mdt, tag="v2t")
            for w0 in range(0, TW, 4):
                ptr = psum_tile()
                for c in range(4):
                    nc.tensor.transpose(ptr[:, 128 * c:128 * (c + 1)],
                                        v2[:, :, w0 + c].bitcast(f32),
                                        ident[:])
                nc.vector.tensor_copy(
                    out=v2t[:, w0:w0 + 4, :].rearrange("p a b -> p (a b)"),
                    in_=ptr[:, :512])

            # attention tail: transpose p, AV, normalize, store
            for g in range(8):
                p_sb, sums = psm[g]
                ptr = psum_tile()
                nc.tensor.transpose(ptr[:, :128], p_sb[:], ident[:])
                pt_sb = pp.tile([128, 128], mdt, tag="ptr")
                nc.vector.tensor_copy(out=pt_sb[:], in_=ptr[:, :128])
                ps_a = psum_tile()
                nc.tensor.matmul(ps_a[:, :512], pt_sb[:],
                                 v2t[:, g * 4:g * 4 + 4, :].rearrange(
                                     "p a b -> p (a b)"),
                                 start=True, stop=True)
                a_sb = pp.tile([128, 128], f32, tag="a")
                for cg in range(4):
                    nc.vector.tensor_scalar_mul(
                        a_sb[32 * cg:32 * (cg + 1), :],
                        ps_a[32 * cg:32 * (cg + 1), 128 * cg:128 * (cg + 1)],
                        sums[32 * cg:32 * (cg + 1), 1:2])
                nc.sync.dma_start(out=a_out[t * 8 + g], in_=a_sb[:])
            yt_cur = yt_next


def build_kernel_b():
    mdt = _mdt()
    nc = bacc.Bacc()
    a_slab = nc.dram_tensor("a_slab", [128, 18, 34], mdt, kind="ExternalInput")
    wp = nc.dram_tensor("wp", [128, 9, 128], mdt, kind="ExternalInput")
    bp = nc.dram_tensor("bp", [128, 1], f32, kind="ExternalInput")
    z_out = nc.dram_tensor("z_out", [128, 512], f32, kind="ExternalOutput")

    with tile.TileContext(nc) as tc:
        with tc.tile_pool(name="sb", bufs=1) as sb, \
             tc.tile_pool(name="ps", bufs=2, space="PSUM") as ps:
            a_sb = sb.tile([128, 18, 34], mdt)
            nc.sync.dma_start(out=a_sb[:], in_=a_slab[:])
            wp_sb = sb.tile([128, 9, 128], mdt)
            nc.sync.dma_start(out=wp_sb[:], in_=wp[:])
            bp_sb = sb.tile([128, 1], f32)
            nc.sync.dma_start(out=bp_sb[:], in_=bp[:])
            pt = ps.tile([128, 512], f32)
            for tap in range(9):
                dy, dx = divmod(tap, 3)
                nc.tensor.matmul(pt[:], wp_sb[:, tap, :],
                                 a_sb[:, dy:dy + 16, dx:dx + 32],
                                 start=(tap == 0), stop=(tap == 8))
            z_sb = sb.tile([128, 512], f32)
            nc.scalar.activation(out=z_sb[:], in_=pt[:], func=AF.Relu,
                                 bias=bp_sb[:, 0:1], scale=1.0)
            nc.sync.dma_start(out=z_out[:], in_=z_sb[:])
    nc.compile()
    return nc


def _round12(a):
    if not USE_F32R:
        return np.ascontiguousarray(a, np.float32)
    b = np.ascontiguousarray(a, np.float32).view(np.uint32)
    b = (b + np.uint32(0x400)) & np.uint32(0xFFFFF800)
    return b.view(np.float32)


def _get_nc(which):
    key = (which, USE_F32R, COLTILE)
    if key not in _NC_CACHE:
        _NC_CACHE[key] = (build_kernel_a() if which == "a"
                          else build_kernel_b())
    return _NC_CACHE[key]


def _prep_core_a(xr, yr, uc, wt, bias6, b, j):
    """Per-core host prep. xr/yr pre-rounded full arrays."""
    y = yr[b]                      # (128, 128, 512)
    x = xr[b]                      # (128, 32, 32)
    u = uc[b].reshape(SS)          # int64 in [0, 512)

    x_slab = np.zeros((128, 34, 34), np.float32)
    x_slab[:, 1:33, 1:33] = x

    y_slab = np.zeros((128, 130, 260), np.float32)
    lo, hi = WH * j - 2, WH * j + WH + 2
    glo, ghi = max(lo, 0), min(hi, WW)
    y_slab[:, 1:129, (glo - lo):(ghi - lo)] = y[:, :, glo:ghi]
    y_tiles = np.stack([y_slab[:, :, TW * t:TW * t + 36]
                        for t in range(NTILE)])

    local = u - WH * j
    mask = (local >= 0) & (local < WH)
    slotq = np.full((NSLOT,), 2000.0, np.float32)
    counts = np.zeros(WH, np.int64)
    for q in range(SS):
        if mask[q]:
            w = int(local[q])
            r = counts[w]
            assert r < M, f"column {w} overflows {M} slots"
            slotq[w * M + r] = float(q)
            counts[w] += 1

    edge = np.ones((128, 16), np.float32)
    if j == 0:
        edge[:, 0] = 0.0        # tile 0, col0 -> global col -1
    else:
        edge[:, 2 * (NTILE - 1) + 1] = 0.0   # last tile col33 -> global 512

    iota8 = (np.arange(8, dtype=np.float32)[None, :] * 128
             + np.arange(128, dtype=np.float32)[:, None])

    in_map = {
        "y_tiles": y_tiles,
        "x_slab": x_slab,
        "bias6": bias6,
        "slotq": slotq.reshape(4, 2048),
        "iota8": iota8,
        "ones1": np.ones((1, 128), np.float32),
        "edge": edge,
    }
    in_map.update(wt)
    return in_map, slotq


def kernel(x, y, u, q_w1, q_b1, q_w2, q_b2, k_w1, k_b1, k_w2, k_b2,
           v_w1, v_b1, v_w2, v_b2, proj_w, proj_b):
    x = np.asarray(x, np.float32)
    y = np.asarray(y, np.float32)
    u_in = np.asarray(u)
    uc = np.clip(u_in, 0, WW - 1).astype(np.int64)

    xr, yr = _round12(x), _round12(y)
    wsrc = {"wq1": q_w1, "wq2": q_w2, "wk1": k_w1, "wk2": k_w2,
            "wv1": v_w1, "wv2": v_w2}
    wt = {n: _round12(np.asarray(w, np.float32)
                      .transpose(1, 2, 3, 0).reshape(128, 9, 128))
          for n, w in wsrc.items()}
    bias6 = np.stack([
        np.asarray(q_b1, np.float32),
        np.asarray(q_b2, np.float32) * np.float32(SCALE),
        np.asarray(k_b1, np.float32), np.asarray(k_b2, np.float32),
        np.asarray(v_b1, np.float32), np.asarray(v_b2, np.float32),
    ], axis=1)                     # (128, 6)

    in_maps, slot_maps = [], []
    for c in range(NCORE):
        im, sq = _prep_core_a(xr, yr, uc, wt, bias6, c // 2, c % 2)
        in_maps.append(im)
        slot_maps.append(sq)

    global LAST_EXEC_NS, LAST_TRACES
    LAST_TRACES = []
    if TRACE:
        _ensure_ntff_hook()
    tkw = dict(trace=True, trace_cores=list(range(NCORE))) if TRACE else {}
    nc_a = _get_nc("a")
    res_a = run_bass_kernel_spmd(nc_a, in_maps, list(range(NCORE)), **tkw)

    a_full = np.zeros((BB, SS, 128), np.float32)
    for c in range(NCORE):
        flat = res_a.results[c]["a_out"].reshape(NSLOT, 128)
        sq = slot_maps[c]
        valid = sq < 1024
        a_full[c // 2][sq[valid].astype(np.int64)] = flat[valid]
    a_img = a_full.transpose(0, 2, 1).reshape(BB, 128, S, S)

    wpr = _round12(np.asarray(proj_w, np.float32)
                   .transpose(1, 2, 3, 0).reshape(128, 9, 128))
    bpr = np.asarray(proj_b, np.float32).reshape(128, 1)
    in_maps_b = []
    for c in range(NCORE):
        b, rh = c // 2, c % 2
        a_slab = np.zeros((128, 18, 34), np.float32)
        r0 = 16 * rh
        rlo, rhi = max(r0 - 1, 0), min(r0 + 17, S)
        a_slab[:, (rlo - (r0 - 1)):(rhi - (r0 - 1)), 1:33] = \
            _round12(a_img[b, :, rlo:rhi, :])
        in_maps_b.append({"a_slab": a_slab, "wp": wpr, "bp": bpr})

    nc_b = _get_nc("b")
    res_b = run_bass_kernel_spmd(nc_b, in_maps_b, list(range(NCORE)), **tkw)
    if TRACE:
        LAST_EXEC_NS = (res_a.exec_time_ns or 0) + (res_b.exec_time_ns or 0)
        LAST_TRACES = [res_a, res_b]

    z = np.zeros((BB, 128, S, S), np.float32)
    for c in range(NCORE):
        b, rh = c // 2, c % 2
        z[b, :, 16 * rh:16 * rh + 16, :] = \
            res_b.results[c]["z_out"].reshape(128, 16, 32)
    return z



# revision 24
# speedup vs baseline: 1.3526x; 1.0500x over previous
"""Trainium2 Bass kernel for nn_CrossAttention (B=4, C=128, S=32, H=128, W=512).

Strategy (8 NeuronCores, SPMD single program):
  core c -> (batch b=c//2, W-half j=c%2).
  Each core: conv_block K/V over its 256-col half of y (streamed in 8
  W-tiles of 32, fused: K/V never touch DRAM), conv_block Q over x
  (duplicated per pair), attention computed per K/V column for the
  queries whose u falls in that column (host assigns queries to
  per-column slots; Q gathered into slot order on device via one-hot
  matmul). Output: per-slot attention vectors; host scatters to the
  query grid. A second tiny kernel applies the final 3x3 proj conv+relu
  (core c -> (batch, 16-row half)).

All heavy matmuls run as float32r (12-bit mantissa, 1 cyc/row at
free-dim>=256) when USE_F32R, else float32 (exact, 4 cyc/row).
"""
import os
import numpy as np

import concourse.tile as tile
from concourse import bacc, mybir
from concourse.bass_utils import run_bass_kernel_spmd
from concourse.masks import make_identity

# ---- problem constants (hardcoded per spec) ----
BB, CC, S = 4, 128, 32
SS = S * S            # 1024 queries/batch
HH, WW = 128, 512
SCALE = float(32 ** -0.5)
TW = 32               # W-tile width
M = 32                # slots per column
WH = 256              # per-core W half
NSLOT = WH * M        # 8192
NTILE = WH // TW      # 8
NCORE = 8

f32 = mybir.dt.float32
f32r = mybir.dt.float32r
bf16 = mybir.dt.bfloat16
AF = mybir.ActivationFunctionType
ALU = mybir.AluOpType
AX = mybir.AxisListType

USE_F32R = True
COLTILE = False       # col-tiled M=32 matmuls with tile_position (if supported)

TRACE = os.environ.get("KERNEL_TRACE", "0") == "1"
LAST_EXEC_NS = None
LAST_TRACES = []


def _ensure_ntff_hook():
    """The container boot can't register the NTFF profile hook because the
    image's antenv lacks axon_hooks; inject an equivalent module so
    run_bass_kernel_spmd(trace=True) works. Trace-path only."""
    import sys
    import types
    try:
        from antenv.axon_hooks import get_axon_ntff_profile_hook  # noqa
        return
    except ImportError:
        pass
    from trn_agent_boot.trn_boot import _ntff_profile_via_ctypes
    hook = _ntff_profile_via_ctypes('/opt/axon/libaxon_pjrt.so')
    mod = types.ModuleType("antenv.axon_hooks")
    mod.get_axon_ntff_profile_hook = lambda: hook
    mod.set_axon_ntff_profile_hook = lambda h: None
    sys.modules["antenv.axon_hooks"] = mod

_NC_CACHE = {}

WNAMES = ["wq1", "wq2", "wk1", "wk2", "wv1", "wv2"]


def _mdt():
    return f32r if USE_F32R else f32


def _conv_groups(rows, r_grp):
    """Split `rows` into groups of r_grp (last may be smaller)."""
    out = []
    r0 = 0
    while r0 < rows:
        out.append((r0, min(r_grp, rows - r0)))
        r0 += r_grp
    return out


def _chunks(lst, n):
    for i in range(0, len(lst), n):
        yield lst[i:i + n]


W_DT = {"wq1": "mdt", "wq2": "mdt", "wk2": "mdt", "wv2": "mdt",
        "wk1": "bf16", "wv1": "bf16"}


def build_kernel_a(reps=1):
    mdt = _mdt()
    wdt = {n: (bf16 if W_DT[n] == "bf16" else mdt) for n in WNAMES}
    nc = bacc.Bacc()

    y_tiles = nc.dram_tensor("y_tiles", [NTILE, 128, 130, 36], bf16,
                             kind="ExternalInput")
    x_slab = nc.dram_tensor("x_slab", [128, 34, 34], mdt, kind="ExternalInput")
    w_dram = {n: nc.dram_tensor(n, [128, 9, 128], wdt[n],
                                kind="ExternalInput")
              for n in WNAMES}
    bias6 = nc.dram_tensor("bias6", [128, 6], f32, kind="ExternalInput")
    slotq = nc.dram_tensor("slotq", [4, 2048], mdt, kind="ExternalInput")
    iota8 = nc.dram_tensor("iota8", [128, 8], f32, kind="ExternalInput")
    ones1 = nc.dram_tensor("ones1", [1, 128], mdt, kind="ExternalInput")
    edge = nc.dram_tensor("edge", [128, 16], f32, kind="ExternalInput")
    a_out = nc.dram_tensor("a_out", [NSLOT // 128, 128, 128], f32,
                           kind="ExternalOutput")

    from contextlib import ExitStack
    with tile.TileContext(nc) as tc, ExitStack() as ctx:
        wpool = ctx.enter_context(tc.tile_pool(name="weights", bufs=1))
        const = ctx.enter_context(tc.tile_pool(name="const", bufs=1))
        qgp = ctx.enter_context(tc.tile_pool(name="qg", bufs=1))
        ps = ctx.enter_context(tc.tile_pool(name="ps", bufs=6, space="PSUM"))
        psb = ctx.enter_context(tc.tile_pool(name="psb", bufs=2,
                                             space="PSUM"))

        def psum_tile():
            return ps.tile([128, 512], f32, tag="ps", name="pst")

        def psum_tile_bf():
            return psb.tile([128, 512], bf16, tag="psb", name="psbt")

        # Q-path critical DMAs first (wq1/wq2 + small consts); K/V weights
        # and edge are deferred until after the first y tile's DMA issues.
        w_sb = {n: wpool.tile([128, 9, 128], wdt[n], tag=n, name=n)
                for n in WNAMES}
        for n in ("wq1", "wq2"):
            nc.sync.dma_start(out=w_sb[n][:], in_=w_dram[n][:])
        bias_sb = const.tile([128, 6], f32, tag="bias")
        nc.sync.dma_start(out=bias_sb[:], in_=bias6[:])
        iota_sb = const.tile([128, 8], f32, tag="iota")
        nc.sync.dma_start(out=iota_sb[:], in_=iota8[:])
        ones_sb = const.tile([1, 128], mdt, tag="ones")
        nc.sync.dma_start(out=ones_sb[:], in_=ones1[:])
        edge_sb = const.tile([128, 16], f32, tag="edge")
        deferred = [(w_sb[n], w_dram[n]) for n in ("wk1", "wk2", "wv1",
                                                   "wv2")]
        deferred.append((edge_sb, edge))
        ident = const.tile([128, 128], f32, tag="ident")
        make_identity(nc, ident[:])
        ident_bf = const.tile([128, 128], bf16, tag="identb")
        nc.vector.tensor_copy(out=ident_bf[:], in_=ident[:])
        zeros34 = const.tile([128, 34], f32, tag="zeros34")
        nc.vector.memset(zeros34[:], 0.0)

        def zfill(ap):
            # zero-fill an f32r view via DVE copy (memset can't emit f32r)
            nc.vector.tensor_copy(out=ap, in_=zeros34[:, :ap.free_size()])

        _build_body(nc, tc, ctx, mdt, psum_tile, psum_tile_bf, w_sb, bias_sb,
                    iota_sb, ones_sb, edge_sb, ident, ident_bf, zfill, qgp,
                    y_tiles, x_slab, slotq, a_out, deferred, reps)
    nc.compile()
    return nc


def _build_body(nc, tc, ctx, mdt, psum_tile, psum_tile_bf, w_sb, bias_sb,
                iota_sb, ones_sb, edge_sb, ident, ident_bf, zfill, qgp,
                y_tiles, x_slab, slotq, a_out, deferred, reps=1):
    if True:
        Qg = qgp.tile([128, NSLOT], bf16, tag="Qg")

        # y tile pool hoisted above the Q path so the first two y DMAs can
        # overlap Q computation (no SBUF WAR against Q-path temporaries).
        yp = ctx.enter_context(tc.tile_pool(name="ytile", bufs=2))
        tile_order = list(range(NTILE)) * reps

        # ---------------- Q path + slot gather ----------------
        with tc.tile_pool(name="qtmp", bufs=1) as qtmp, \
             tc.tile_pool(name="qt", bufs=1) as qtp, \
             tc.tile_pool(name="msel", bufs=2) as mselp, \
             tc.tile_pool(name="bcast", bufs=2) as bcp:
            xq = qtmp.tile([128, 34, 34], mdt, tag="xq")
            nc.sync.dma_start(out=xq[:], in_=x_slab[:])
            yt_cur = yp.tile([128, 130, 36], bf16, tag="yt")
            nc.sync.dma_start(out=yt_cur[:], in_=y_tiles[tile_order[0]])
            for t_sb, t_dram in deferred:
                nc.sync.dma_start(out=t_sb[:], in_=t_dram[:])
            q1 = qtmp.tile([128, 34, 34], mdt, tag="q1")
            q1f = q1[:].rearrange("p a b -> p (a b)")
            zfill(q1f[:, 0:34])                              # row 0
            zfill(q1f[:, 33 * 34:34 * 34])                   # row 33
            zfill(q1[:, :, 0:1].rearrange("p a b -> p (a b)"))    # col 0
            zfill(q1[:, :, 33:34].rearrange("p a b -> p (a b)"))  # col 33
            # conv1-Q: valid 32x32 -> q1[1:33, 1:33]
            for r0 in (0, 16):
                pt = psum_tile()
                for tap in range(9):
                    dy, dx = divmod(tap, 3)
                    nc.tensor.matmul(pt[:, :512], w_sb["wq1"][:, tap, :],
                                     xq[:, r0 + dy:r0 + dy + 16, dx:dx + 32],
                                     start=(tap == 0), stop=(tap == 8))
                # rows 1+r0 .. 1+r0+16 of q1, cols 1..33 (strided out)
                nc.scalar.activation(out=q1[:, 1 + r0:1 + r0 + 16, 1:33],
                                     in_=pt[:, :512].rearrange(
                                         "p (a b) -> p a b", a=16),
                                     func=AF.Relu, bias=bias_sb[:, 0:1],
                                     scale=1.0)
            # conv2-Q -> Q (128, 1024) f32, scaled by SCALE
            q2 = qtmp.tile([128, 1024], f32, tag="q2")
            for r0 in (0, 16):
                pt = psum_tile()
                for tap in range(9):
                    dy, dx = divmod(tap, 3)
                    nc.tensor.matmul(pt[:, :512], w_sb["wq2"][:, tap, :],
                                     q1[:, r0 + dy:r0 + dy + 16, dx:dx + 32],
                                     start=(tap == 0), stop=(tap == 8))
                nc.scalar.activation(out=q2[:, r0 * 32:(r0 + 16) * 32],
                                     in_=pt[:, :512], func=AF.Identity,
                                     bias=bias_sb[:, 1:2], scale=SCALE)
            # transpose Q -> 8 chunks (q, e)
            qt_all = qtp.tile([128, 8, 128], bf16, tag="qt")
            for qc in range(8):
                pt = psum_tile()
                nc.tensor.transpose(pt[:, :128],
                                    q2[:, qc * 128:(qc + 1) * 128], ident[:])
                nc.scalar.copy(out=qt_all[:, qc, :], in_=pt[:, :128])
            # build Qg = Q gathered into slots, via one-hot matmuls
            for blk in range(4):
                sq = bcp.tile([1, 2048], mdt, tag="sq")
                nc.sync.dma_start(out=sq[:], in_=slotq[blk:blk + 1, :])
                bc = bcp.tile([128, 2048], f32, tag="bc")
                for nch in range(4):
                    pb = psum_tile()
                    nc.tensor.matmul(pb[:, :512], ones_sb[:],
                                     sq[:, nch * 512:(nch + 1) * 512],
                                     start=True, stop=True)
                    nc.scalar.copy(out=bc[:, nch * 512:(nch + 1) * 512],
                                   in_=pb[:, :512])
                pq = [psum_tile() for _ in range(4)]
                for qc in range(8):
                    ms = mselp.tile([128, 2048], bf16, tag="ms")
                    nc.vector.tensor_scalar(out=ms[:], in0=bc[:],
                                            scalar1=iota_sb[:, qc:qc + 1],
                                            scalar2=None, op0=ALU.is_equal)
                    for nch in range(4):
                        nc.tensor.matmul(pq[nch][:, :512], qt_all[:, qc, :],
                                         ms[:, nch * 512:(nch + 1) * 512],
                                         start=(qc == 0), stop=(qc == 7))
                for nch in range(4):
                    off = blk * 2048 + nch * 512
                    nc.scalar.copy(out=Qg[:, off:off + 512],
                                   in_=pq[nch][:, :512])

        # ---------------- main loop over W-tiles ----------------
        c1p = ctx.enter_context(tc.tile_pool(name="c1", bufs=1))
        k2p = ctx.enter_context(tc.tile_pool(name="k2", bufs=1))
        v2p = ctx.enter_context(tc.tile_pool(name="v2", bufs=1))
        v2tp = ctx.enter_context(tc.tile_pool(name="v2t", bufs=2))
        pp = ctx.enter_context(tc.tile_pool(name="pwork", bufs=2))
        pbank = ctx.enter_context(tc.tile_pool(name="pbank", bufs=1))
        smp = ctx.enter_context(tc.tile_pool(name="smax", bufs=1))

        g1 = _conv_groups(128, 15)   # conv1 valid rows (h 0..127)
        g2 = _conv_groups(128, 16)   # conv2 rows

        def conv1(w1n, b1i, t, yt):
            c1 = c1p.tile([128, 130, 34], mdt, tag="c1")
            c1f = c1[:].rearrange("p a b -> p (a b)")
            zfill(c1f[:, 0:34])
            zfill(c1f[:, 129 * 34:130 * 34])
            for ci, sg in enumerate(_chunks(g1, 4)):
                pts = [psum_tile() for _ in sg]
                for tap in range(9):
                    dy, dx = divmod(tap, 3)
                    for (r0, R), pt in zip(sg, pts):
                        nc.tensor.matmul(
                            pt[:, :R * 34], w_sb[w1n][:, tap, :],
                            yt[:, r0 + dy:r0 + dy + R, dx:dx + 34],
                            start=(tap == 0), stop=(tap == 8))
                for (r0, R), pt in zip(sg, pts):
                    nc.scalar.activation(
                        out=c1f[:, (1 + r0) * 34:(1 + r0 + R) * 34],
                        in_=pt[:, :R * 34], func=AF.Relu,
                        bias=bias_sb[:, b1i:b1i + 1], scale=1.0)
                yield ci
            # zero conv1 halo cols outside the global image (data mask)
            nc.vector.tensor_scalar_mul(c1[:, :, 0:1], c1[:, :, 0:1],
                                        edge_sb[:, 2 * t:2 * t + 1])
            nc.vector.tensor_scalar_mul(c1[:, :, 33:34], c1[:, :, 33:34],
                                        edge_sb[:, 2 * t + 1:2 * t + 2])
            yield c1

        def conv2(w2n, b2i, c1, pool2, kind):
            # K is stored (e, w, h) so a 4-column slab is one contiguous
            # 512-wide moving operand for the batched QK matmul. Evictions
            # alternate Scalar/Vector so the serial chain before QK halves.
            if kind == "K":
                cv2 = pool2.tile([128, 32, 128], bf16, tag="cv2K")
            else:
                cv2 = pool2.tile([128, 128, 32], bf16, tag="cv2V")
                cv2f = cv2[:].rearrange("p a b -> p (a b)")
            ei = 0
            for sg in _chunks(g2, 4):
                pts = [psum_tile() for _ in sg]
                for tap in range(9):
                    dy, dx = divmod(tap, 3)
                    for (r0, R), pt in zip(sg, pts):
                        nc.tensor.matmul(
                            pt[:, :R * 32], w_sb[w2n][:, tap, :],
                            c1[:, r0 + dy:r0 + dy + R, dx:dx + 32],
                            start=(tap == 0), stop=(tap == 8))
                for (r0, R), pt in zip(sg, pts):
                    if kind == "K":
                        out_ap = cv2[:, :, r0:r0 + R].transpose([0, 2, 1])
                        in_ap = pt[:, :R * 32].rearrange(
                            "p (a b) -> p a b", a=R)
                    else:
                        out_ap = cv2f[:, r0 * 32:(r0 + R) * 32]
                        in_ap = pt[:, :R * 32]
                    if ei % 2 == 0:
                        nc.scalar.activation(
                            out=out_ap, in_=in_ap, func=AF.Identity,
                            bias=bias_sb[:, b2i:b2i + 1], scale=1.0)
                    else:
                        nc.vector.tensor_scalar(
                            out=out_ap, in0=in_ap,
                            scalar1=bias_sb[:, b2i:b2i + 1],
                            scalar2=None, op0=ALU.add)
                    ei += 1
            return cv2

        def qk_softmax(t, g, k2):
            # scores for 128 slots (4 columns x 32 slots) in one matmul:
            # out[slot, c*128+h]; then exp of the 4 diagonal blocks.
            slot0 = (t * TW + g * 4) * M
            ps_s = psum_tile()
            nc.tensor.matmul(ps_s[:, :512], Qg[:, slot0:slot0 + 128],
                             k2[:, g * 4:g * 4 + 4, :].rearrange(
                                 "p a b -> p (a b)"),
                             start=True, stop=True)
            p_sb = pbank.tile([128, 128], bf16, tag=f"p{g}")
            for cg in range(4):
                nc.scalar.activation(
                    out=p_sb[32 * cg:32 * (cg + 1), :],
                    in_=ps_s[32 * cg:32 * (cg + 1), 128 * cg:128 * (cg + 1)],
                    func=AF.Exp, bias=0.0, scale=1.0)
            sums = smp.tile([128, 2], f32, tag=f"sm{g}")
            nc.vector.tensor_reduce(out=sums[:, 0:1], in_=p_sb[:],
                                    axis=AX.X, op=ALU.add)
            nc.vector.reciprocal(out=sums[:, 1:2], in_=sums[:, 0:1])
            return p_sb, sums

        for ti, t in enumerate(tile_order):
            yt = yt_cur
            if ti + 1 < len(tile_order):
                yt_next = yp.tile([128, 130, 36], bf16, tag="yt")
                nc.sync.dma_start(out=yt_next[:],
                                  in_=y_tiles[tile_order[ti + 1]])
            else:
                yt_next = None

            # K path
            ck = conv1("wk1", 2, t, yt)
            for _ in range(3):
                next(ck)
            c1 = next(ck)
            k2 = conv2("wk2", 3, c1, k2p, "K")

            # QK + softmax interleaved with the V path for PSUM slack
            psm = [None] * 8
            for g in range(4):
                psm[g] = qk_softmax(t, g, k2)
            cv = conv1("wv1", 4, t, yt)
            next(cv)
            for g in range(4, 8):
                psm[g] = qk_softmax(t, g, k2)
            next(cv)
            next(cv)
            c1 = next(cv)
            v2 = conv2("wv2", 5, c1, v2p, "V")

            # V2T: per-column PE transpose -> (h, w, e), 4 columns per bank
            v2t = v2tp.tile([128, 32, 128], bf16, tag="v2t")
            for w0 in range(0, TW, 4):
                ptr = psum_tile_bf()
                for c in range(4):
                    nc.tensor.transpose(ptr[:, 128 * c:128 * (c + 1)],
                                        v2[:, :, w0 + c], ident_bf[:])
                nc.vector.tensor_copy(
                    out=v2t[:, w0:w0 + 4, :].rearrange("p a b -> p (a b)"),
                    in_=ptr[:, :512])

            # attention tail: transpose p, AV, normalize, store
            for g in range(8):
                p_sb, sums = psm[g]
                ptr = psum_tile_bf()
                nc.tensor.transpose(ptr[:, :128], p_sb[:], ident_bf[:])
                pt_sb = pp.tile([128, 128], bf16, tag="ptr")
                nc.vector.tensor_copy(out=pt_sb[:], in_=ptr[:, :128])
                ps_a = psum_tile()
                nc.tensor.matmul(ps_a[:, :512], pt_sb[:],
                                 v2t[:, g * 4:g * 4 + 4, :].rearrange(
                                     "p a b -> p (a b)"),
                                 start=True, stop=True)
                a_sb = pp.tile([128, 128], f32, tag="a")
                for cg in range(4):
                    nc.vector.tensor_scalar_mul(
                        a_sb[32 * cg:32 * (cg + 1), :],
                        ps_a[32 * cg:32 * (cg + 1), 128 * cg:128 * (cg + 1)],
                        sums[32 * cg:32 * (cg + 1), 1:2])
                nc.sync.dma_start(out=a_out[t * 8 + g], in_=a_sb[:])
            yt_cur = yt_next


def build_kernel_b():
    mdt = _mdt()
    nc = bacc.Bacc()
    a_slab = nc.dram_tensor("a_slab", [128, 18, 34], mdt, kind="ExternalInput")
    wp = nc.dram_tensor("wp", [128, 9, 128], mdt, kind="ExternalInput")
    bp = nc.dram_tensor("bp", [128, 1], f32, kind="ExternalInput")
    z_out = nc.dram_tensor("z_out", [128, 512], f32, kind="ExternalOutput")

    with tile.TileContext(nc) as tc:
        with tc.tile_pool(name="sb", bufs=1) as sb, \
             tc.tile_pool(name="ps", bufs=2, space="PSUM") as ps:
            a_sb = sb.tile([128, 18, 34], mdt)
            nc.sync.dma_start(out=a_sb[:], in_=a_slab[:])
            wp_sb = sb.tile([128, 9, 128], mdt)
            nc.sync.dma_start(out=wp_sb[:], in_=wp[:])
            bp_sb = sb.tile([128, 1], f32)
            nc.sync.dma_start(out=bp_sb[:], in_=bp[:])
            pt = ps.tile([128, 512], f32)
            for tap in range(9):
                dy, dx = divmod(tap, 3)
                nc.tensor.matmul(pt[:], wp_sb[:, tap, :],
                                 a_sb[:, dy:dy + 16, dx:dx + 32],
                                 start=(tap == 0), stop=(tap == 8))
            z_sb = sb.tile([128, 512], f32)
            nc.scalar.activation(out=z_sb[:], in_=pt[:], func=AF.Relu,
                                 bias=bp_sb[:, 0:1], scale=1.0)
            nc.sync.dma_start(out=z_out[:], in_=z_sb[:])
    nc.compile()
    return nc


def _round12(a):
    if not USE_F32R:
        return np.ascontiguousarray(a, np.float32)
    b = np.ascontiguousarray(a, np.float32).view(np.uint32)
    b = (b + np.uint32(0x400)) & np.uint32(0xFFFFF800)
    return b.view(np.float32)


def _get_nc(which):
    key = (which, USE_F32R, COLTILE)
    if key not in _NC_CACHE:
        _NC_CACHE[key] = (build_kernel_a() if which == "a"
                          else build_kernel_b())
    return _NC_CACHE[key]


def _prep_core_a(xr, yr, uc, wt, bias6, b, j):
    """Per-core host prep. xr/yr pre-rounded full arrays."""
    import ml_dtypes
    y = yr[b]                      # (128, 128, 512)
    x = xr[b]                      # (128, 32, 32)
    u = uc[b].reshape(SS)          # int64 in [0, 512)

    x_slab = np.zeros((128, 34, 34), np.float32)
    x_slab[:, 1:33, 1:33] = x

    y_slab = np.zeros((128, 130, 260), ml_dtypes.bfloat16)
    lo, hi = WH * j - 2, WH * j + WH + 2
    glo, ghi = max(lo, 0), min(hi, WW)
    y_slab[:, 1:129, (glo - lo):(ghi - lo)] = y[:, :, glo:ghi].astype(
        ml_dtypes.bfloat16)
    y_tiles = np.stack([y_slab[:, :, TW * t:TW * t + 36]
                        for t in range(NTILE)])

    local = u - WH * j
    mask = (local >= 0) & (local < WH)
    slotq = np.full((NSLOT,), 2000.0, np.float32)
    counts = np.zeros(WH, np.int64)
    for q in range(SS):
        if mask[q]:
            w = int(local[q])
            r = counts[w]
            assert r < M, f"column {w} overflows {M} slots"
            slotq[w * M + r] = float(q)
            counts[w] += 1

    edge = np.ones((128, 16), np.float32)
    if j == 0:
        edge[:, 0] = 0.0        # tile 0, col0 -> global col -1
    else:
        edge[:, 2 * (NTILE - 1) + 1] = 0.0   # last tile col33 -> global 512

    iota8 = (np.arange(8, dtype=np.float32)[None, :] * 128
             + np.arange(128, dtype=np.float32)[:, None])

    in_map = {
        "y_tiles": y_tiles,
        "x_slab": x_slab,
        "bias6": bias6,
        "slotq": slotq.reshape(4, 2048),
        "iota8": iota8,
        "ones1": np.ones((1, 128), np.float32),
        "edge": edge,
    }
    in_map.update(wt)
    return in_map, slotq


def kernel(x, y, u, q_w1, q_b1, q_w2, q_b2, k_w1, k_b1, k_w2, k_b2,
           v_w1, v_b1, v_w2, v_b2, proj_w, proj_b):
    x = np.asarray(x, np.float32)
    y = np.asarray(y, np.float32)
    u_in = np.asarray(u)
    uc = np.clip(u_in, 0, WW - 1).astype(np.int64)

    import ml_dtypes
    xr, yr = _round12(x), y    # y is cast to bf16 in _prep_core_a
    wsrc = {"wq1": q_w1, "wq2": q_w2, "wk1": k_w1, "wk2": k_w2,
            "wv1": v_w1, "wv2": v_w2}

    def _prep_w(n, w):
        wr = np.asarray(w, np.float32).transpose(1, 2, 3, 0).reshape(
            128, 9, 128)
        if W_DT[n] == "bf16":
            return np.ascontiguousarray(wr).astype(ml_dtypes.bfloat16)
        return _round12(wr)

    wt = {n: _prep_w(n, w) for n, w in wsrc.items()}
    bias6 = np.stack([
        np.asarray(q_b1, np.float32),
        np.asarray(q_b2, np.float32) * np.float32(SCALE),
        np.asarray(k_b1, np.float32), np.asarray(k_b2, np.float32),
        np.asarray(v_b1, np.float32), np.asarray(v_b2, np.float32),
    ], axis=1)                     # (128, 6)

    in_maps, slot_maps = [], []
    for c in range(NCORE):
        im, sq = _prep_core_a(xr, yr, uc, wt, bias6, c // 2, c % 2)
        in_maps.append(im)
        slot_maps.append(sq)

    global LAST_EXEC_NS, LAST_TRACES
    LAST_TRACES = []
    if TRACE:
        _ensure_ntff_hook()
    tkw = dict(trace=True, trace_cores=list(range(NCORE))) if TRACE else {}
    nc_a = _get_nc("a")
    res_a = run_bass_kernel_spmd(nc_a, in_maps, list(range(NCORE)), **tkw)

    a_full = np.zeros((BB, SS, 128), np.float32)
    for c in range(NCORE):
        flat = res_a.results[c]["a_out"].reshape(NSLOT, 128)
        sq = slot_maps[c]
        valid = sq < 1024
        a_full[c // 2][sq[valid].astype(np.int64)] = flat[valid]
    a_img = a_full.transpose(0, 2, 1).reshape(BB, 128, S, S)

    wpr = _round12(np.asarray(proj_w, np.float32)
                   .transpose(1, 2, 3, 0).reshape(128, 9, 128))
    bpr = np.asarray(proj_b, np.float32).reshape(128, 1)
    in_maps_b = []
    for c in range(NCORE):
        b, rh = c // 2, c % 2
        a_slab = np.zeros((128, 18, 34), np.float32)
        r0 = 16 * rh
        rlo, rhi = max(r0 - 1, 0), min(r0 + 17, S)
        a_slab[:, (rlo - (r0 - 1)):(rhi - (r0 - 1)), 1:33] = \
            _round12(a_img[b, :, rlo:rhi, :])
        in_maps_b.append({"a_slab": a_slab, "wp": wpr, "bp": bpr})

    nc_b = _get_nc("b")
    res_b = run_bass_kernel_spmd(nc_b, in_maps_b, list(range(NCORE)), **tkw)
    if TRACE:
        LAST_EXEC_NS = (res_a.exec_time_ns or 0) + (res_b.exec_time_ns or 0)
        LAST_TRACES = [res_a, res_b]

    z = np.zeros((BB, 128, S, S), np.float32)
    for c in range(NCORE):
        b, rh = c // 2, c % 2
        z[b, :, 16 * rh:16 * rh + 16, :] = \
            res_b.results[c]["z_out"].reshape(128, 16, 32)
    return z



# revision 26
# speedup vs baseline: 1.3551x; 1.0019x over previous
"""Trainium2 Bass kernel for nn_CrossAttention (B=4, C=128, S=32, H=128, W=512).

Strategy (8 NeuronCores, SPMD single program):
  core c -> (batch b=c//2, W-half j=c%2).
  Each core: conv_block K/V over its 256-col half of y (streamed in 8
  W-tiles of 32, fused: K/V never touch DRAM), conv_block Q over x
  (duplicated per pair), attention computed per K/V column for the
  queries whose u falls in that column (host assigns queries to
  per-column slots; Q gathered into slot order on device via one-hot
  matmul). Output: per-slot attention vectors; host scatters to the
  query grid. A second tiny kernel applies the final 3x3 proj conv+relu
  (core c -> (batch, 16-row half)).

All heavy matmuls run as float32r (12-bit mantissa, 1 cyc/row at
free-dim>=256) when USE_F32R, else float32 (exact, 4 cyc/row).
"""
import os
import numpy as np

import concourse.tile as tile
from concourse import bacc, mybir
from concourse.bass_utils import run_bass_kernel_spmd
from concourse.masks import make_identity

# ---- problem constants (hardcoded per spec) ----
BB, CC, S = 4, 128, 32
SS = S * S            # 1024 queries/batch
HH, WW = 128, 512
SCALE = float(32 ** -0.5)
TW = 32               # W-tile width
M = 32                # slots per column
WH = 256              # per-core W half
NSLOT = WH * M        # 8192
NTILE = WH // TW      # 8
NCORE = 8

f32 = mybir.dt.float32
f32r = mybir.dt.float32r
bf16 = mybir.dt.bfloat16
AF = mybir.ActivationFunctionType
ALU = mybir.AluOpType
AX = mybir.AxisListType

USE_F32R = True
COLTILE = False       # col-tiled M=32 matmuls with tile_position (if supported)

TRACE = os.environ.get("KERNEL_TRACE", "0") == "1"
LAST_EXEC_NS = None
LAST_TRACES = []


def _ensure_ntff_hook():
    """The container boot can't register the NTFF profile hook because the
    image's antenv lacks axon_hooks; inject an equivalent module so
    run_bass_kernel_spmd(trace=True) works. Trace-path only."""
    import sys
    import types
    try:
        from antenv.axon_hooks import get_axon_ntff_profile_hook  # noqa
        return
    except ImportError:
        pass
    from trn_agent_boot.trn_boot import _ntff_profile_via_ctypes
    hook = _ntff_profile_via_ctypes('/opt/axon/libaxon_pjrt.so')
    mod = types.ModuleType("antenv.axon_hooks")
    mod.get_axon_ntff_profile_hook = lambda: hook
    mod.set_axon_ntff_profile_hook = lambda h: None
    sys.modules["antenv.axon_hooks"] = mod

_NC_CACHE = {}

WNAMES = ["wq1", "wq2", "wk1", "wk2", "wv1", "wv2"]


def _mdt():
    return f32r if USE_F32R else f32


def _conv_groups(rows, r_grp):
    """Split `rows` into groups of r_grp (last may be smaller)."""
    out = []
    r0 = 0
    while r0 < rows:
        out.append((r0, min(r_grp, rows - r0)))
        r0 += r_grp
    return out


def _chunks(lst, n):
    for i in range(0, len(lst), n):
        yield lst[i:i + n]


W_DT = {"wq1": "mdt", "wq2": "mdt", "wk2": "mdt", "wv2": "mdt",
        "wk1": "bf16", "wv1": "bf16"}


def build_kernel_a(reps=1):
    mdt = _mdt()
    wdt = {n: (bf16 if W_DT[n] == "bf16" else mdt) for n in WNAMES}
    nc = bacc.Bacc()

    y_tiles = nc.dram_tensor("y_tiles", [NTILE, 128, 130, 36], bf16,
                             kind="ExternalInput")
    x_slab = nc.dram_tensor("x_slab", [128, 34, 34], mdt, kind="ExternalInput")
    w_dram = {n: nc.dram_tensor(n, [128, 9, 128], wdt[n],
                                kind="ExternalInput")
              for n in WNAMES}
    bias6 = nc.dram_tensor("bias6", [128, 6], f32, kind="ExternalInput")
    slotq = nc.dram_tensor("slotq", [4, 2048], mdt, kind="ExternalInput")
    iota8 = nc.dram_tensor("iota8", [128, 8], f32, kind="ExternalInput")
    ones1 = nc.dram_tensor("ones1", [1, 128], mdt, kind="ExternalInput")
    edge = nc.dram_tensor("edge", [128, 16], f32, kind="ExternalInput")
    a_out = nc.dram_tensor("a_out", [NSLOT // 128, 128, 128], f32,
                           kind="ExternalOutput")

    from contextlib import ExitStack
    with tile.TileContext(nc) as tc, ExitStack() as ctx:
        wpool = ctx.enter_context(tc.tile_pool(name="weights", bufs=1))
        const = ctx.enter_context(tc.tile_pool(name="const", bufs=1))
        qgp = ctx.enter_context(tc.tile_pool(name="qg", bufs=1))
        ps = ctx.enter_context(tc.tile_pool(name="ps", bufs=6, space="PSUM"))
        psb = ctx.enter_context(tc.tile_pool(name="psb", bufs=2,
                                             space="PSUM"))

        def psum_tile():
            return ps.tile([128, 512], f32, tag="ps", name="pst")

        def psum_tile_bf():
            return psb.tile([128, 512], bf16, tag="psb", name="psbt")

        # Q-path critical DMAs first (wq1/wq2 + small consts); K/V weights
        # and edge are deferred until after the first y tile's DMA issues.
        w_sb = {n: wpool.tile([128, 9, 128], wdt[n], tag=n, name=n)
                for n in WNAMES}
        for n in ("wq1", "wq2"):
            nc.sync.dma_start(out=w_sb[n][:], in_=w_dram[n][:])
        bias_sb = const.tile([128, 6], f32, tag="bias")
        nc.sync.dma_start(out=bias_sb[:], in_=bias6[:])
        iota_sb = const.tile([128, 8], f32, tag="iota")
        nc.sync.dma_start(out=iota_sb[:], in_=iota8[:])
        ones_sb = const.tile([1, 128], mdt, tag="ones")
        nc.sync.dma_start(out=ones_sb[:], in_=ones1[:])
        edge_sb = const.tile([128, 16], f32, tag="edge")
        deferred = [(w_sb[n], w_dram[n]) for n in ("wk1", "wk2", "wv1",
                                                   "wv2")]
        deferred.append((edge_sb, edge))
        ident = const.tile([128, 128], f32, tag="ident")
        make_identity(nc, ident[:])
        ident_bf = const.tile([128, 128], bf16, tag="identb")
        nc.vector.tensor_copy(out=ident_bf[:], in_=ident[:])
        zeros34 = const.tile([128, 34], f32, tag="zeros34")
        nc.vector.memset(zeros34[:], 0.0)

        def zfill(ap):
            # zero-fill an f32r view via DVE copy (memset can't emit f32r)
            nc.vector.tensor_copy(out=ap, in_=zeros34[:, :ap.free_size()])

        _build_body(nc, tc, ctx, mdt, psum_tile, psum_tile_bf, w_sb, bias_sb,
                    iota_sb, ones_sb, edge_sb, ident, ident_bf, zfill, qgp,
                    y_tiles, x_slab, slotq, a_out, deferred, reps)
    nc.compile()
    return nc


def _build_body(nc, tc, ctx, mdt, psum_tile, psum_tile_bf, w_sb, bias_sb,
                iota_sb, ones_sb, edge_sb, ident, ident_bf, zfill, qgp,
                y_tiles, x_slab, slotq, a_out, deferred, reps=1):
    if True:
        Qg = qgp.tile([128, NSLOT], bf16, tag="Qg")

        # y tile pool hoisted above the Q path so the first two y DMAs can
        # overlap Q computation (no SBUF WAR against Q-path temporaries).
        yp = ctx.enter_context(tc.tile_pool(name="ytile", bufs=2))
        tile_order = list(range(NTILE)) * reps

        # ---------------- Q path + slot gather ----------------
        with tc.tile_pool(name="qtmp", bufs=1) as qtmp, \
             tc.tile_pool(name="qt", bufs=1) as qtp, \
             tc.tile_pool(name="msel", bufs=2) as mselp, \
             tc.tile_pool(name="bcast", bufs=2) as bcp:
            xq = qtmp.tile([128, 34, 34], mdt, tag="xq")
            nc.sync.dma_start(out=xq[:], in_=x_slab[:])
            yt_cur = yp.tile([128, 130, 36], bf16, tag="yt")
            nc.sync.dma_start(out=yt_cur[:], in_=y_tiles[tile_order[0]])
            for t_sb, t_dram in deferred:
                nc.sync.dma_start(out=t_sb[:], in_=t_dram[:])
            q1 = qtmp.tile([128, 34, 34], mdt, tag="q1")
            q1f = q1[:].rearrange("p a b -> p (a b)")
            zfill(q1f[:, 0:34])                              # row 0
            zfill(q1f[:, 33 * 34:34 * 34])                   # row 33
            zfill(q1[:, :, 0:1].rearrange("p a b -> p (a b)"))    # col 0
            zfill(q1[:, :, 33:34].rearrange("p a b -> p (a b)"))  # col 33
            # conv1-Q: valid 32x32 -> q1[1:33, 1:33]
            for r0 in (0, 16):
                pt = psum_tile()
                for tap in range(9):
                    dy, dx = divmod(tap, 3)
                    nc.tensor.matmul(pt[:, :512], w_sb["wq1"][:, tap, :],
                                     xq[:, r0 + dy:r0 + dy + 16, dx:dx + 32],
                                     start=(tap == 0), stop=(tap == 8))
                # rows 1+r0 .. 1+r0+16 of q1, cols 1..33 (strided out)
                nc.scalar.activation(out=q1[:, 1 + r0:1 + r0 + 16, 1:33],
                                     in_=pt[:, :512].rearrange(
                                         "p (a b) -> p a b", a=16),
                                     func=AF.Relu, bias=bias_sb[:, 0:1],
                                     scale=1.0)
            # conv2-Q -> Q (128, 1024) f32, scaled by SCALE
            q2 = qtmp.tile([128, 1024], f32, tag="q2")
            for r0 in (0, 16):
                pt = psum_tile()
                for tap in range(9):
                    dy, dx = divmod(tap, 3)
                    nc.tensor.matmul(pt[:, :512], w_sb["wq2"][:, tap, :],
                                     q1[:, r0 + dy:r0 + dy + 16, dx:dx + 32],
                                     start=(tap == 0), stop=(tap == 8))
                nc.scalar.activation(out=q2[:, r0 * 32:(r0 + 16) * 32],
                                     in_=pt[:, :512], func=AF.Identity,
                                     bias=bias_sb[:, 1:2], scale=SCALE)
            # transpose Q -> 8 chunks (q, e)
            qt_all = qtp.tile([128, 8, 128], bf16, tag="qt")
            for qc in range(8):
                pt = psum_tile()
                nc.tensor.transpose(pt[:, :128],
                                    q2[:, qc * 128:(qc + 1) * 128], ident[:])
                nc.scalar.copy(out=qt_all[:, qc, :], in_=pt[:, :128])
            # build Qg = Q gathered into slots, via one-hot matmuls
            for blk in range(4):
                sq = bcp.tile([1, 2048], mdt, tag="sq")
                nc.sync.dma_start(out=sq[:], in_=slotq[blk:blk + 1, :])
                bc = bcp.tile([128, 2048], f32, tag="bc")
                for nch in range(4):
                    pb = psum_tile()
                    nc.tensor.matmul(pb[:, :512], ones_sb[:],
                                     sq[:, nch * 512:(nch + 1) * 512],
                                     start=True, stop=True)
                    nc.scalar.copy(out=bc[:, nch * 512:(nch + 1) * 512],
                                   in_=pb[:, :512])
                pq = [psum_tile() for _ in range(4)]
                for qc in range(8):
                    ms = mselp.tile([128, 2048], bf16, tag="ms")
                    nc.vector.tensor_scalar(out=ms[:], in0=bc[:],
                                            scalar1=iota_sb[:, qc:qc + 1],
                                            scalar2=None, op0=ALU.is_equal)
                    for nch in range(4):
                        nc.tensor.matmul(pq[nch][:, :512], qt_all[:, qc, :],
                                         ms[:, nch * 512:(nch + 1) * 512],
                                         start=(qc == 0), stop=(qc == 7))
                for nch in range(4):
                    off = blk * 2048 + nch * 512
                    nc.scalar.copy(out=Qg[:, off:off + 512],
                                   in_=pq[nch][:, :512])

        # ---------------- main loop over W-tiles ----------------
        c1p = ctx.enter_context(tc.tile_pool(name="c1", bufs=1))
        k2p = ctx.enter_context(tc.tile_pool(name="k2", bufs=1))
        v2p = ctx.enter_context(tc.tile_pool(name="v2", bufs=1))
        v2tp = ctx.enter_context(tc.tile_pool(name="v2t", bufs=2))
        pp = ctx.enter_context(tc.tile_pool(name="pwork", bufs=2))
        pbank = ctx.enter_context(tc.tile_pool(name="pbank", bufs=1))
        smp = ctx.enter_context(tc.tile_pool(name="smax", bufs=1))

        g1 = _conv_groups(128, 15)   # conv1 valid rows (h 0..127)
        g2 = _conv_groups(128, 16)   # conv2 rows

        def conv1(w1n, b1i, t, yt):
            c1 = c1p.tile([128, 130, 34], mdt, tag="c1")
            c1f = c1[:].rearrange("p a b -> p (a b)")
            zfill(c1f[:, 0:34])
            zfill(c1f[:, 129 * 34:130 * 34])
            for ci, sg in enumerate(_chunks(g1, 4)):
                pts = [psum_tile() for _ in sg]
                for tap in range(9):
                    dy, dx = divmod(tap, 3)
                    for (r0, R), pt in zip(sg, pts):
                        nc.tensor.matmul(
                            pt[:, :R * 34], w_sb[w1n][:, tap, :],
                            yt[:, r0 + dy:r0 + dy + R, dx:dx + 34],
                            start=(tap == 0), stop=(tap == 8))
                for (r0, R), pt in zip(sg, pts):
                    nc.scalar.activation(
                        out=c1f[:, (1 + r0) * 34:(1 + r0 + R) * 34],
                        in_=pt[:, :R * 34], func=AF.Relu,
                        bias=bias_sb[:, b1i:b1i + 1], scale=1.0)
                yield ci
            # zero conv1 halo cols outside the global image (data mask)
            nc.vector.tensor_scalar_mul(c1[:, :, 0:1], c1[:, :, 0:1],
                                        edge_sb[:, 2 * t:2 * t + 1])
            nc.vector.tensor_scalar_mul(c1[:, :, 33:34], c1[:, :, 33:34],
                                        edge_sb[:, 2 * t + 1:2 * t + 2])
            yield c1

        def conv2(w2n, b2i, c1, pool2, kind):
            # contiguous (e, h, w) evictions, alternating Scalar/Vector so
            # the serial chain before the QK consumer halves.
            cv2 = pool2.tile([128, 128, 32], bf16, tag="cv2" + kind)
            cv2f = cv2[:].rearrange("p a b -> p (a b)")
            ei = 0
            for sg in _chunks(g2, 4):
                pts = [psum_tile() for _ in sg]
                for tap in range(9):
                    dy, dx = divmod(tap, 3)
                    for (r0, R), pt in zip(sg, pts):
                        nc.tensor.matmul(
                            pt[:, :R * 32], w_sb[w2n][:, tap, :],
                            c1[:, r0 + dy:r0 + dy + R, dx:dx + 32],
                            start=(tap == 0), stop=(tap == 8))
                for (r0, R), pt in zip(sg, pts):
                    out_ap = cv2f[:, r0 * 32:(r0 + R) * 32]
                    in_ap = pt[:, :R * 32]
                    if ei % 2 == 0:
                        nc.scalar.activation(
                            out=out_ap, in_=in_ap, func=AF.Identity,
                            bias=bias_sb[:, b2i:b2i + 1], scale=1.0)
                    else:
                        nc.vector.tensor_scalar(
                            out=out_ap, in0=in_ap,
                            scalar1=bias_sb[:, b2i:b2i + 1],
                            scalar2=None, op0=ALU.add)
                    ei += 1
            return cv2

        def qk_softmax(t, g, k2):
            # scores for 128 slots (4 columns x 32 slots) in one matmul:
            # out[slot, c*128+h]; then exp of the 4 diagonal blocks.
            slot0 = (t * TW + g * 4) * M
            ps_s = psum_tile()
            nc.tensor.matmul(ps_s[:, :512], Qg[:, slot0:slot0 + 128],
                             k2[:, :, g * 4:g * 4 + 4].transpose([0, 2, 1]),
                             start=True, stop=True)
            p_sb = pbank.tile([128, 128], bf16, tag=f"p{g}")
            for cg in range(4):
                nc.scalar.activation(
                    out=p_sb[32 * cg:32 * (cg + 1), :],
                    in_=ps_s[32 * cg:32 * (cg + 1), 128 * cg:128 * (cg + 1)],
                    func=AF.Exp, bias=0.0, scale=1.0)
            sums = smp.tile([128, 2], f32, tag=f"sm{g}")
            nc.vector.tensor_reduce(out=sums[:, 0:1], in_=p_sb[:],
                                    axis=AX.X, op=ALU.add)
            nc.vector.reciprocal(out=sums[:, 1:2], in_=sums[:, 0:1])
            return p_sb, sums

        for ti, t in enumerate(tile_order):
            yt = yt_cur
            if ti + 1 < len(tile_order):
                yt_next = yp.tile([128, 130, 36], bf16, tag="yt")
                nc.sync.dma_start(out=yt_next[:],
                                  in_=y_tiles[tile_order[ti + 1]])
            else:
                yt_next = None

            # K path
            ck = conv1("wk1", 2, t, yt)
            for _ in range(3):
                next(ck)
            c1 = next(ck)
            k2 = conv2("wk2", 3, c1, k2p, "K")

            # QK + softmax interleaved with the V path for PSUM slack
            psm = [None] * 8
            for g in range(4):
                psm[g] = qk_softmax(t, g, k2)
            cv = conv1("wv1", 4, t, yt)
            next(cv)
            for g in range(4, 8):
                psm[g] = qk_softmax(t, g, k2)
            next(cv)
            next(cv)
            c1 = next(cv)
            v2 = conv2("wv2", 5, c1, v2p, "V")

            # V2T: per-column PE transpose -> (h, w, e), 4 columns per bank
            v2t = v2tp.tile([128, 32, 128], bf16, tag="v2t")
            for w0 in range(0, TW, 4):
                ptr = psum_tile_bf()
                for c in range(4):
                    nc.tensor.transpose(ptr[:, 128 * c:128 * (c + 1)],
                                        v2[:, :, w0 + c], ident_bf[:])
                nc.vector.tensor_copy(
                    out=v2t[:, w0:w0 + 4, :].rearrange("p a b -> p (a b)"),
                    in_=ptr[:, :512])

            # attention tail: transpose p, AV, normalize, store
            for g in range(8):
                p_sb, sums = psm[g]
                ptr = psum_tile_bf()
                nc.tensor.transpose(ptr[:, :128], p_sb[:], ident_bf[:])
                pt_sb = pp.tile([128, 128], bf16, tag="ptr")
                nc.vector.tensor_copy(out=pt_sb[:], in_=ptr[:, :128])
                ps_a = psum_tile()
                nc.tensor.matmul(ps_a[:, :512], pt_sb[:],
                                 v2t[:, g * 4:g * 4 + 4, :].rearrange(
                                     "p a b -> p (a b)"),
                                 start=True, stop=True)
                a_sb = pp.tile([128, 128], f32, tag="a")
                for cg in range(4):
                    nc.vector.tensor_scalar_mul(
                        a_sb[32 * cg:32 * (cg + 1), :],
                        ps_a[32 * cg:32 * (cg + 1), 128 * cg:128 * (cg + 1)],
                        sums[32 * cg:32 * (cg + 1), 1:2])
                nc.sync.dma_start(out=a_out[t * 8 + g], in_=a_sb[:])
            yt_cur = yt_next


def build_kernel_b():
    mdt = _mdt()
    nc = bacc.Bacc()
    a_slab = nc.dram_tensor("a_slab", [128, 18, 34], mdt, kind="ExternalInput")
    wp = nc.dram_tensor("wp", [128, 9, 128], mdt, kind="ExternalInput")
    bp = nc.dram_tensor("bp", [128, 1], f32, kind="ExternalInput")
    z_out = nc.dram_tensor("z_out", [128, 512], f32, kind="ExternalOutput")

    with tile.TileContext(nc) as tc:
        with tc.tile_pool(name="sb", bufs=1) as sb, \
             tc.tile_pool(name="ps", bufs=2, space="PSUM") as ps:
            a_sb = sb.tile([128, 18, 34], mdt)
            nc.sync.dma_start(out=a_sb[:], in_=a_slab[:])
            wp_sb = sb.tile([128, 9, 128], mdt)
            nc.sync.dma_start(out=wp_sb[:], in_=wp[:])
            bp_sb = sb.tile([128, 1], f32)
            nc.sync.dma_start(out=bp_sb[:], in_=bp[:])
            pt = ps.tile([128, 512], f32)
            for tap in range(9):
                dy, dx = divmod(tap, 3)
                nc.tensor.matmul(pt[:], wp_sb[:, tap, :],
                                 a_sb[:, dy:dy + 16, dx:dx + 32],
                                 start=(tap == 0), stop=(tap == 8))
            z_sb = sb.tile([128, 512], f32)
            nc.scalar.activation(out=z_sb[:], in_=pt[:], func=AF.Relu,
                                 bias=bp_sb[:, 0:1], scale=1.0)
            nc.sync.dma_start(out=z_out[:], in_=z_sb[:])
    nc.compile()
    return nc


def _round12(a):
    if not USE_F32R:
        return np.ascontiguousarray(a, np.float32)
    b = np.ascontiguousarray(a, np.float32).view(np.uint32)
    b = (b + np.uint32(0x400)) & np.uint32(0xFFFFF800)
    return b.view(np.float32)


def _get_nc(which):
    key = (which, USE_F32R, COLTILE)
    if key not in _NC_CACHE:
        _NC_CACHE[key] = (build_kernel_a() if which == "a"
                          else build_kernel_b())
    return _NC_CACHE[key]


def _prep_core_a(xr, yr, uc, wt, bias6, b, j):
    """Per-core host prep. xr/yr pre-rounded full arrays."""
    import ml_dtypes
    y = yr[b]                      # (128, 128, 512)
    x = xr[b]                      # (128, 32, 32)
    u = uc[b].reshape(SS)          # int64 in [0, 512)

    x_slab = np.zeros((128, 34, 34), np.float32)
    x_slab[:, 1:33, 1:33] = x

    y_slab = np.zeros((128, 130, 260), ml_dtypes.bfloat16)
    lo, hi = WH * j - 2, WH * j + WH + 2
    glo, ghi = max(lo, 0), min(hi, WW)
    y_slab[:, 1:129, (glo - lo):(ghi - lo)] = y[:, :, glo:ghi].astype(
        ml_dtypes.bfloat16)
    y_tiles = np.stack([y_slab[:, :, TW * t:TW * t + 36]
                        for t in range(NTILE)])

    local = u - WH * j
    mask = (local >= 0) & (local < WH)
    slotq = np.full((NSLOT,), 2000.0, np.float32)
    counts = np.zeros(WH, np.int64)
    for q in range(SS):
        if mask[q]:
            w = int(local[q])
            r = counts[w]
            assert r < M, f"column {w} overflows {M} slots"
            slotq[w * M + r] = float(q)
            counts[w] += 1

    edge = np.ones((128, 16), np.float32)
    if j == 0:
        edge[:, 0] = 0.0        # tile 0, col0 -> global col -1
    else:
        edge[:, 2 * (NTILE - 1) + 1] = 0.0   # last tile col33 -> global 512

    iota8 = (np.arange(8, dtype=np.float32)[None, :] * 128
             + np.arange(128, dtype=np.float32)[:, None])

    in_map = {
        "y_tiles": y_tiles,
        "x_slab": x_slab,
        "bias6": bias6,
        "slotq": slotq.reshape(4, 2048),
        "iota8": iota8,
        "ones1": np.ones((1, 128), np.float32),
        "edge": edge,
    }
    in_map.update(wt)
    return in_map, slotq


def kernel(x, y, u, q_w1, q_b1, q_w2, q_b2, k_w1, k_b1, k_w2, k_b2,
           v_w1, v_b1, v_w2, v_b2, proj_w, proj_b):
    x = np.asarray(x, np.float32)
    y = np.asarray(y, np.float32)
    u_in = np.asarray(u)
    uc = np.clip(u_in, 0, WW - 1).astype(np.int64)

    import ml_dtypes
    xr, yr = _round12(x), y    # y is cast to bf16 in _prep_core_a
    wsrc = {"wq1": q_w1, "wq2": q_w2, "wk1": k_w1, "wk2": k_w2,
            "wv1": v_w1, "wv2": v_w2}

    def _prep_w(n, w):
        wr = np.asarray(w, np.float32).transpose(1, 2, 3, 0).reshape(
            128, 9, 128)
        if W_DT[n] == "bf16":
            return np.ascontiguousarray(wr).astype(ml_dtypes.bfloat16)
        return _round12(wr)

    wt = {n: _prep_w(n, w) for n, w in wsrc.items()}
    bias6 = np.stack([
        np.asarray(q_b1, np.float32),
        np.asarray(q_b2, np.float32) * np.float32(SCALE),
        np.asarray(k_b1, np.float32), np.asarray(k_b2, np.float32),
        np.asarray(v_b1, np.float32), np.asarray(v_b2, np.float32),
    ], axis=1)                     # (128, 6)

    in_maps, slot_maps = [], []
    for c in range(NCORE):
        im, sq = _prep_core_a(xr, yr, uc, wt, bias6, c // 2, c % 2)
        in_maps.append(im)
        slot_maps.append(sq)

    global LAST_EXEC_NS, LAST_TRACES
    LAST_TRACES = []
    if TRACE:
        _ensure_ntff_hook()
    tkw = dict(trace=True, trace_cores=list(range(NCORE))) if TRACE else {}
    nc_a = _get_nc("a")
    res_a = run_bass_kernel_spmd(nc_a, in_maps, list(range(NCORE)), **tkw)

    a_full = np.zeros((BB, SS, 128), np.float32)
    for c in range(NCORE):
        flat = res_a.results[c]["a_out"].reshape(NSLOT, 128)
        sq = slot_maps[c]
        valid = sq < 1024
        a_full[c // 2][sq[valid].astype(np.int64)] = flat[valid]
    a_img = a_full.transpose(0, 2, 1).reshape(BB, 128, S, S)

    wpr = _round12(np.asarray(proj_w, np.float32)
                   .transpose(1, 2, 3, 0).reshape(128, 9, 128))
    bpr = np.asarray(proj_b, np.float32).reshape(128, 1)
    in_maps_b = []
    for c in range(NCORE):
        b, rh = c // 2, c % 2
        a_slab = np.zeros((128, 18, 34), np.float32)
        r0 = 16 * rh
        rlo, rhi = max(r0 - 1, 0), min(r0 + 17, S)
        a_slab[:, (rlo - (r0 - 1)):(rhi - (r0 - 1)), 1:33] = \
            _round12(a_img[b, :, rlo:rhi, :])
        in_maps_b.append({"a_slab": a_slab, "wp": wpr, "bp": bpr})

    nc_b = _get_nc("b")
    res_b = run_bass_kernel_spmd(nc_b, in_maps_b, list(range(NCORE)), **tkw)
    if TRACE:
        LAST_EXEC_NS = (res_a.exec_time_ns or 0) + (res_b.exec_time_ns or 0)
        LAST_TRACES = [res_a, res_b]

    z = np.zeros((BB, 128, S, S), np.float32)
    for c in range(NCORE):
        b, rh = c // 2, c % 2
        z[b, :, 16 * rh:16 * rh + 16, :] = \
            res_b.results[c]["z_out"].reshape(128, 16, 32)
    return z



# revision 30
# speedup vs baseline: 1.5156x; 1.1184x over previous
"""Trainium2 Bass kernel for nn_CrossAttention (B=4, C=128, S=32, H=128, W=512).

Strategy (8 NeuronCores, SPMD single program):
  core c -> (batch b=c//2, W-half j=c%2).
  Each core: conv_block K/V over its 256-col half of y (streamed in 8
  W-tiles of 32, fused: K/V never touch DRAM), conv_block Q over x
  (duplicated per pair), attention computed per K/V column for the
  queries whose u falls in that column (host assigns queries to
  per-column slots; Q gathered into slot order on device via one-hot
  matmul). Output: per-slot attention vectors; host scatters to the
  query grid. A second tiny kernel applies the final 3x3 proj conv+relu
  (core c -> (batch, 16-row half)).

All heavy matmuls run as float32r (12-bit mantissa, 1 cyc/row at
free-dim>=256) when USE_F32R, else float32 (exact, 4 cyc/row).
"""
import os
import numpy as np

import concourse.tile as tile
from concourse import bacc, mybir
from concourse.bass_utils import run_bass_kernel_spmd
from concourse.masks import make_identity

# ---- problem constants (hardcoded per spec) ----
BB, CC, S = 4, 128, 32
SS = S * S            # 1024 queries/batch
HH, WW = 128, 512
SCALE = float(32 ** -0.5)
TW = 32               # W-tile width
M = 32                # slots per column
WH = 256              # per-core W half
NSLOT = WH * M        # 8192
NTILE = WH // TW      # 8
NCORE = 8

f32 = mybir.dt.float32
f32r = mybir.dt.float32r
bf16 = mybir.dt.bfloat16
AF = mybir.ActivationFunctionType
ALU = mybir.AluOpType
AX = mybir.AxisListType

USE_F32R = True
COLTILE = False       # col-tiled M=32 matmuls with tile_position (if supported)

TRACE = os.environ.get("KERNEL_TRACE", "0") == "1"
LAST_EXEC_NS = None
LAST_TRACES = []


def _ensure_ntff_hook():
    """The container boot can't register the NTFF profile hook because the
    image's antenv lacks axon_hooks; inject an equivalent module so
    run_bass_kernel_spmd(trace=True) works. Trace-path only."""
    import sys
    import types
    try:
        from antenv.axon_hooks import get_axon_ntff_profile_hook  # noqa
        return
    except ImportError:
        pass
    from trn_agent_boot.trn_boot import _ntff_profile_via_ctypes
    hook = _ntff_profile_via_ctypes('/opt/axon/libaxon_pjrt.so')
    mod = types.ModuleType("antenv.axon_hooks")
    mod.get_axon_ntff_profile_hook = lambda: hook
    mod.set_axon_ntff_profile_hook = lambda h: None
    sys.modules["antenv.axon_hooks"] = mod

_NC_CACHE = {}

WNAMES = ["wq1", "wq2", "wk1", "wk2", "wv1", "wv2"]


def _mdt():
    return f32r if USE_F32R else f32


def _conv_groups(rows, r_grp):
    """Split `rows` into groups of r_grp (last may be smaller)."""
    out = []
    r0 = 0
    while r0 < rows:
        out.append((r0, min(r_grp, rows - r0)))
        r0 += r_grp
    return out


def _chunks(lst, n):
    for i in range(0, len(lst), n):
        yield lst[i:i + n]


W_DT = {"wq1": "mdt", "wq2": "mdt", "wk2": "mdt", "wv2": "mdt",
        "wk1": "bf16", "wv1": "bf16"}


def build_kernel_a(reps=1):
    mdt = _mdt()
    wdt = {n: (bf16 if W_DT[n] == "bf16" else mdt) for n in WNAMES}
    nc = bacc.Bacc()

    y_tiles = nc.dram_tensor("y_tiles", [NTILE, 128, 130, 36], bf16,
                             kind="ExternalInput")
    x_slab = nc.dram_tensor("x_slab", [128, 34, 34], mdt, kind="ExternalInput")
    w_dram = {n: nc.dram_tensor(n, [128, 9, 128], wdt[n],
                                kind="ExternalInput")
              for n in WNAMES}
    bias6 = nc.dram_tensor("bias6", [128, 6], f32, kind="ExternalInput")
    slotq = nc.dram_tensor("slotq", [4, 2048], mdt, kind="ExternalInput")
    iota8 = nc.dram_tensor("iota8", [128, 8], f32, kind="ExternalInput")
    ones1 = nc.dram_tensor("ones1", [1, 128], mdt, kind="ExternalInput")
    edge = nc.dram_tensor("edge", [128, 16], f32, kind="ExternalInput")
    a_out = nc.dram_tensor("a_out", [NSLOT // 128, 128, 128], f32,
                           kind="ExternalOutput")

    from contextlib import ExitStack
    with tile.TileContext(nc) as tc, ExitStack() as ctx:
        wpool = ctx.enter_context(tc.tile_pool(name="weights", bufs=1))
        const = ctx.enter_context(tc.tile_pool(name="const", bufs=1))
        qgp = ctx.enter_context(tc.tile_pool(name="qg", bufs=1))
        ps = ctx.enter_context(tc.tile_pool(name="ps", bufs=6, space="PSUM"))
        psb = ctx.enter_context(tc.tile_pool(name="psb", bufs=2,
                                             space="PSUM"))

        def psum_tile():
            return ps.tile([128, 512], f32, tag="ps", name="pst")

        def psum_tile_bf():
            return psb.tile([128, 512], bf16, tag="psb", name="psbt")

        # Q-path critical DMAs first (wq1/wq2 + small consts); K/V weights
        # and edge are deferred until after the first y tile's DMA issues.
        w_sb = {n: wpool.tile([128, 9, 128], wdt[n], tag=n, name=n)
                for n in WNAMES}
        for n in ("wq1", "wq2"):
            nc.sync.dma_start(out=w_sb[n][:], in_=w_dram[n][:])
        bias_sb = const.tile([128, 6], f32, tag="bias")
        nc.sync.dma_start(out=bias_sb[:], in_=bias6[:])
        iota_sb = const.tile([128, 8], f32, tag="iota")
        nc.sync.dma_start(out=iota_sb[:], in_=iota8[:])
        ones_sb = const.tile([1, 128], mdt, tag="ones")
        nc.sync.dma_start(out=ones_sb[:], in_=ones1[:])
        edge_sb = const.tile([128, 16], f32, tag="edge")
        deferred = [(w_sb[n], w_dram[n]) for n in ("wk1", "wk2", "wv1",
                                                   "wv2")]
        deferred.append((edge_sb, edge))
        ident = const.tile([128, 128], f32, tag="ident")
        make_identity(nc, ident[:])
        ident_bf = const.tile([128, 128], bf16, tag="identb")
        nc.vector.tensor_copy(out=ident_bf[:], in_=ident[:])
        zeros34 = const.tile([128, 34], f32, tag="zeros34")
        nc.vector.memset(zeros34[:], 0.0)

        def zfill(ap):
            # zero-fill an f32r view via DVE copy (memset can't emit f32r)
            nc.vector.tensor_copy(out=ap, in_=zeros34[:, :ap.free_size()])

        _build_body(nc, tc, ctx, mdt, psum_tile, psum_tile_bf, w_sb, bias_sb,
                    iota_sb, ones_sb, edge_sb, ident, ident_bf, zfill, qgp,
                    y_tiles, x_slab, slotq, a_out, deferred, reps)
    nc.compile()
    return nc


def _build_body(nc, tc, ctx, mdt, psum_tile, psum_tile_bf, w_sb, bias_sb,
                iota_sb, ones_sb, edge_sb, ident, ident_bf, zfill, qgp,
                y_tiles, x_slab, slotq, a_out, deferred, reps=1):
    if True:
        Qg = qgp.tile([128, NSLOT], bf16, tag="Qg")

        # y tile pool hoisted above the Q path so the first two y DMAs can
        # overlap Q computation (no SBUF WAR against Q-path temporaries).
        yp = ctx.enter_context(tc.tile_pool(name="ytile", bufs=2))
        tile_order = list(range(NTILE)) * reps

        # ---------------- Q path + slot gather ----------------
        with tc.tile_pool(name="qtmp", bufs=1) as qtmp, \
             tc.tile_pool(name="qt", bufs=1) as qtp, \
             tc.tile_pool(name="msel", bufs=2) as mselp, \
             tc.tile_pool(name="bcast", bufs=2) as bcp:
            xq = qtmp.tile([128, 34, 34], mdt, tag="xq")
            nc.sync.dma_start(out=xq[:], in_=x_slab[:])
            yt_cur = yp.tile([128, 130, 36], bf16, tag="yt")
            nc.sync.dma_start(out=yt_cur[:], in_=y_tiles[tile_order[0]])
            for t_sb, t_dram in deferred:
                nc.sync.dma_start(out=t_sb[:], in_=t_dram[:])
            q1 = qtmp.tile([128, 34, 34], mdt, tag="q1")
            q1f = q1[:].rearrange("p a b -> p (a b)")
            zfill(q1f[:, 0:34])                              # row 0
            zfill(q1f[:, 33 * 34:34 * 34])                   # row 33
            zfill(q1[:, :, 0:1].rearrange("p a b -> p (a b)"))    # col 0
            zfill(q1[:, :, 33:34].rearrange("p a b -> p (a b)"))  # col 33
            # conv1-Q: valid 32x32 -> q1[1:33, 1:33]
            for r0 in (0, 16):
                pt = psum_tile()
                for tap in range(9):
                    dy, dx = divmod(tap, 3)
                    nc.tensor.matmul(pt[:, :512], w_sb["wq1"][:, tap, :],
                                     xq[:, r0 + dy:r0 + dy + 16, dx:dx + 32],
                                     start=(tap == 0), stop=(tap == 8))
                # rows 1+r0 .. 1+r0+16 of q1, cols 1..33 (strided out)
                nc.scalar.activation(out=q1[:, 1 + r0:1 + r0 + 16, 1:33],
                                     in_=pt[:, :512].rearrange(
                                         "p (a b) -> p a b", a=16),
                                     func=AF.Relu, bias=bias_sb[:, 0:1],
                                     scale=1.0)
            # conv2-Q -> Q (128, 1024) f32, scaled by SCALE
            q2 = qtmp.tile([128, 1024], f32, tag="q2")
            for r0 in (0, 16):
                pt = psum_tile()
                for tap in range(9):
                    dy, dx = divmod(tap, 3)
                    nc.tensor.matmul(pt[:, :512], w_sb["wq2"][:, tap, :],
                                     q1[:, r0 + dy:r0 + dy + 16, dx:dx + 32],
                                     start=(tap == 0), stop=(tap == 8))
                nc.scalar.activation(out=q2[:, r0 * 32:(r0 + 16) * 32],
                                     in_=pt[:, :512], func=AF.Identity,
                                     bias=bias_sb[:, 1:2], scale=SCALE)
            # transpose Q -> 8 chunks (q, e)
            qt_all = qtp.tile([128, 8, 128], bf16, tag="qt")
            for qc in range(8):
                pt = psum_tile()
                nc.tensor.transpose(pt[:, :128],
                                    q2[:, qc * 128:(qc + 1) * 128], ident[:])
                nc.scalar.copy(out=qt_all[:, qc, :], in_=pt[:, :128])
            # build Qg = Q gathered into slots, via one-hot matmuls
            for blk in range(4):
                sq = bcp.tile([1, 2048], mdt, tag="sq")
                nc.sync.dma_start(out=sq[:], in_=slotq[blk:blk + 1, :])
                bc = bcp.tile([128, 2048], f32, tag="bc")
                for nch in range(4):
                    pb = psum_tile()
                    nc.tensor.matmul(pb[:, :512], ones_sb[:],
                                     sq[:, nch * 512:(nch + 1) * 512],
                                     start=True, stop=True)
                    nc.scalar.copy(out=bc[:, nch * 512:(nch + 1) * 512],
                                   in_=pb[:, :512])
                pq = [psum_tile() for _ in range(4)]
                for qc in range(8):
                    ms = mselp.tile([128, 2048], bf16, tag="ms")
                    nc.vector.tensor_scalar(out=ms[:], in0=bc[:],
                                            scalar1=iota_sb[:, qc:qc + 1],
                                            scalar2=None, op0=ALU.is_equal)
                    for nch in range(4):
                        nc.tensor.matmul(pq[nch][:, :512], qt_all[:, qc, :],
                                         ms[:, nch * 512:(nch + 1) * 512],
                                         start=(qc == 0), stop=(qc == 7))
                for nch in range(4):
                    off = blk * 2048 + nch * 512
                    nc.scalar.copy(out=Qg[:, off:off + 512],
                                   in_=pq[nch][:, :512])

        # ---------------- main loop over W-tiles ----------------
        c1p = ctx.enter_context(tc.tile_pool(name="c1", bufs=1))
        k2p = ctx.enter_context(tc.tile_pool(name="k2", bufs=1))
        v2p = ctx.enter_context(tc.tile_pool(name="v2", bufs=1))
        v2tp = ctx.enter_context(tc.tile_pool(name="v2t", bufs=2))
        pp = ctx.enter_context(tc.tile_pool(name="pwork", bufs=2))
        pbank = ctx.enter_context(tc.tile_pool(name="pbank", bufs=1))
        smp = ctx.enter_context(tc.tile_pool(name="smax", bufs=1))

        g1 = _conv_groups(128, 15)   # conv1 valid rows (h 0..127)
        g2 = _conv_groups(128, 16)   # conv2 rows

        def conv1(w1n, b1i, t, yt):
            c1 = c1p.tile([128, 130, 34], mdt, tag="c1")
            c1f = c1[:].rearrange("p a b -> p (a b)")
            zfill(c1f[:, 0:34])
            zfill(c1f[:, 129 * 34:130 * 34])
            ei = 0
            for ci, sg in enumerate(_chunks(g1, 4)):
                pts = [psum_tile() for _ in sg]
                for tap in range(9):
                    dy, dx = divmod(tap, 3)
                    for (r0, R), pt in zip(sg, pts):
                        nc.tensor.matmul(
                            pt[:, :R * 34], w_sb[w1n][:, tap, :],
                            yt[:, r0 + dy:r0 + dy + R, dx:dx + 34],
                            start=(tap == 0), stop=(tap == 8))
                for (r0, R), pt in zip(sg, pts):
                    out_ap = c1f[:, (1 + r0) * 34:(1 + r0 + R) * 34]
                    if ei % 2 == 0:
                        nc.scalar.activation(
                            out=out_ap, in_=pt[:, :R * 34], func=AF.Relu,
                            bias=bias_sb[:, b1i:b1i + 1], scale=1.0)
                    else:
                        nc.vector.tensor_scalar(
                            out=out_ap, in0=pt[:, :R * 34],
                            scalar1=bias_sb[:, b1i:b1i + 1], scalar2=0.0,
                            op0=ALU.add, op1=ALU.max)
                    ei += 1
                yield ci
            # zero conv1 halo cols outside the global image (data mask);
            # only boundary tiles can fall outside, interior halos are real.
            if t == 0:
                nc.vector.tensor_scalar_mul(c1[:, :, 0:1], c1[:, :, 0:1],
                                            edge_sb[:, 0:1])
            if t == NTILE - 1:
                nc.vector.tensor_scalar_mul(
                    c1[:, :, 33:34], c1[:, :, 33:34],
                    edge_sb[:, 2 * t + 1:2 * t + 2])
            yield c1

        def conv2(w2n, b2i, c1, pool2, kind):
            # contiguous (e, h, w) evictions, alternating Scalar/Vector so
            # the serial chain before the QK consumer halves.
            cv2 = pool2.tile([128, 128, 32], bf16, tag="cv2" + kind)
            cv2f = cv2[:].rearrange("p a b -> p (a b)")
            ei = 0
            for sg in _chunks(g2, 4):
                pts = [psum_tile() for _ in sg]
                for tap in range(9):
                    dy, dx = divmod(tap, 3)
                    for (r0, R), pt in zip(sg, pts):
                        nc.tensor.matmul(
                            pt[:, :R * 32], w_sb[w2n][:, tap, :],
                            c1[:, r0 + dy:r0 + dy + R, dx:dx + 32],
                            start=(tap == 0), stop=(tap == 8))
                for (r0, R), pt in zip(sg, pts):
                    out_ap = cv2f[:, r0 * 32:(r0 + R) * 32]
                    in_ap = pt[:, :R * 32]
                    if ei % 2 == 0:
                        nc.scalar.activation(
                            out=out_ap, in_=in_ap, func=AF.Identity,
                            bias=bias_sb[:, b2i:b2i + 1], scale=1.0)
                    else:
                        nc.vector.tensor_scalar(
                            out=out_ap, in0=in_ap,
                            scalar1=bias_sb[:, b2i:b2i + 1],
                            scalar2=None, op0=ALU.add)
                    ei += 1
            return cv2

        def qk_softmax(t, g, k2):
            # scores for 128 slots (4 columns x 32 slots) in one matmul.
            # Moving operand is the natural (h, w) slab so the free order is
            # interleaved: out[slot, h*4+c]; exp of the 4 diagonal blocks
            # reads psum with stride 4.
            slot0 = (t * TW + g * 4) * M
            ps_s = psum_tile()
            nc.tensor.matmul(ps_s[:, :512], Qg[:, slot0:slot0 + 128],
                             k2[:, :, g * 4:g * 4 + 4],
                             start=True, stop=True)
            p_sb = pbank.tile([128, 128], bf16, tag=f"p{g}")
            for cg in range(4):
                nc.scalar.activation(
                    out=p_sb[32 * cg:32 * (cg + 1), :],
                    in_=ps_s[32 * cg:32 * (cg + 1), cg:512:4],
                    func=AF.Exp, bias=0.0, scale=1.0)
            sums = smp.tile([128, 2], f32, tag=f"sm{g}")
            nc.vector.tensor_reduce(out=sums[:, 0:1], in_=p_sb[:],
                                    axis=AX.X, op=ALU.add)
            nc.vector.reciprocal(out=sums[:, 1:2], in_=sums[:, 0:1])
            return p_sb, sums

        for ti, t in enumerate(tile_order):
            yt = yt_cur
            if ti + 1 < len(tile_order):
                yt_next = yp.tile([128, 130, 36], bf16, tag="yt")
                nc.sync.dma_start(out=yt_next[:],
                                  in_=y_tiles[tile_order[ti + 1]])
            else:
                yt_next = None

            # K path
            ck = conv1("wk1", 2, t, yt)
            for _ in range(3):
                next(ck)
            c1 = next(ck)
            k2 = conv2("wk2", 3, c1, k2p, "K")

            # QK + softmax interleaved with the V path for PSUM slack
            psm = [None] * 8
            for g in range(4):
                psm[g] = qk_softmax(t, g, k2)
            cv = conv1("wv1", 4, t, yt)
            next(cv)
            for g in range(4, 8):
                psm[g] = qk_softmax(t, g, k2)
            next(cv)
            next(cv)
            c1 = next(cv)
            v2 = conv2("wv2", 5, c1, v2p, "V")

            # V2T: per-column PE transpose -> (h, w, e), 4 columns per bank
            v2t = v2tp.tile([128, 32, 128], bf16, tag="v2t")
            for w0 in range(0, TW, 4):
                ptr = psum_tile_bf()
                for c in range(4):
                    nc.tensor.transpose(ptr[:, 128 * c:128 * (c + 1)],
                                        v2[:, :, w0 + c], ident_bf[:])
                nc.vector.tensor_copy(
                    out=v2t[:, w0:w0 + 4, :].rearrange("p a b -> p (a b)"),
                    in_=ptr[:, :512])

            # attention tail: transpose p, AV, normalize, store.
            # Normalizing multiplies alternate Scalar/Vector (the tail is
            # otherwise Vector-bound while Scalar idles).
            for g in range(8):
                p_sb, sums = psm[g]
                ptr = psum_tile_bf()
                nc.tensor.transpose(ptr[:, :128], p_sb[:], ident_bf[:])
                pt_sb = pp.tile([128, 128], bf16, tag="ptr")
                nc.scalar.copy(out=pt_sb[:], in_=ptr[:, :128])
                ps_a = psum_tile()
                nc.tensor.matmul(ps_a[:, :512], pt_sb[:],
                                 v2t[:, g * 4:g * 4 + 4, :].rearrange(
                                     "p a b -> p (a b)"),
                                 start=True, stop=True)
                a_sb = pp.tile([128, 128], f32, tag="a")
                for cg in range(4):
                    src = ps_a[32 * cg:32 * (cg + 1),
                               128 * cg:128 * (cg + 1)]
                    dst = a_sb[32 * cg:32 * (cg + 1), :]
                    rc = sums[32 * cg:32 * (cg + 1), 1:2]
                    if cg % 2 == 0:
                        nc.scalar.activation(out=dst, in_=src,
                                             func=AF.Identity, bias=0.0,
                                             scale=rc)
                    else:
                        nc.vector.tensor_scalar_mul(dst, src, rc)
                nc.sync.dma_start(out=a_out[t * 8 + g], in_=a_sb[:])
            yt_cur = yt_next


def build_kernel_b():
    mdt = _mdt()
    nc = bacc.Bacc()
    a_slab = nc.dram_tensor("a_slab", [128, 18, 34], mdt, kind="ExternalInput")
    wp = nc.dram_tensor("wp", [128, 9, 128], mdt, kind="ExternalInput")
    bp = nc.dram_tensor("bp", [128, 1], f32, kind="ExternalInput")
    z_out = nc.dram_tensor("z_out", [128, 512], f32, kind="ExternalOutput")

    with tile.TileContext(nc) as tc:
        with tc.tile_pool(name="sb", bufs=1) as sb, \
             tc.tile_pool(name="ps", bufs=2, space="PSUM") as ps:
            a_sb = sb.tile([128, 18, 34], mdt)
            nc.sync.dma_start(out=a_sb[:], in_=a_slab[:])
            wp_sb = sb.tile([128, 9, 128], mdt)
            nc.sync.dma_start(out=wp_sb[:], in_=wp[:])
            bp_sb = sb.tile([128, 1], f32)
            nc.sync.dma_start(out=bp_sb[:], in_=bp[:])
            pt = ps.tile([128, 512], f32)
            for tap in range(9):
                dy, dx = divmod(tap, 3)
                nc.tensor.matmul(pt[:], wp_sb[:, tap, :],
                                 a_sb[:, dy:dy + 16, dx:dx + 32],
                                 start=(tap == 0), stop=(tap == 8))
            z_sb = sb.tile([128, 512], f32)
            nc.scalar.activation(out=z_sb[:], in_=pt[:], func=AF.Relu,
                                 bias=bp_sb[:, 0:1], scale=1.0)
            nc.sync.dma_start(out=z_out[:], in_=z_sb[:])
    nc.compile()
    return nc


def _round12(a):
    if not USE_F32R:
        return np.ascontiguousarray(a, np.float32)
    b = np.ascontiguousarray(a, np.float32).view(np.uint32)
    b = (b + np.uint32(0x400)) & np.uint32(0xFFFFF800)
    return b.view(np.float32)


def _get_nc(which):
    key = (which, USE_F32R, COLTILE)
    if key not in _NC_CACHE:
        _NC_CACHE[key] = (build_kernel_a() if which == "a"
                          else build_kernel_b())
    return _NC_CACHE[key]


def _prep_core_a(xr, yr, uc, wt, bias6, b, j):
    """Per-core host prep. xr/yr pre-rounded full arrays."""
    import ml_dtypes
    y = yr[b]                      # (128, 128, 512)
    x = xr[b]                      # (128, 32, 32)
    u = uc[b].reshape(SS)          # int64 in [0, 512)

    x_slab = np.zeros((128, 34, 34), np.float32)
    x_slab[:, 1:33, 1:33] = x

    y_slab = np.zeros((128, 130, 260), ml_dtypes.bfloat16)
    lo, hi = WH * j - 2, WH * j + WH + 2
    glo, ghi = max(lo, 0), min(hi, WW)
    y_slab[:, 1:129, (glo - lo):(ghi - lo)] = y[:, :, glo:ghi].astype(
        ml_dtypes.bfloat16)
    y_tiles = np.stack([y_slab[:, :, TW * t:TW * t + 36]
                        for t in range(NTILE)])

    local = u - WH * j
    mask = (local >= 0) & (local < WH)
    slotq = np.full((NSLOT,), 2000.0, np.float32)
    counts = np.zeros(WH, np.int64)
    for q in range(SS):
        if mask[q]:
            w = int(local[q])
            r = counts[w]
            assert r < M, f"column {w} overflows {M} slots"
            slotq[w * M + r] = float(q)
            counts[w] += 1

    edge = np.ones((128, 16), np.float32)
    if j == 0:
        edge[:, 0] = 0.0        # tile 0, col0 -> global col -1
    else:
        edge[:, 2 * (NTILE - 1) + 1] = 0.0   # last tile col33 -> global 512

    iota8 = (np.arange(8, dtype=np.float32)[None, :] * 128
             + np.arange(128, dtype=np.float32)[:, None])

    in_map = {
        "y_tiles": y_tiles,
        "x_slab": x_slab,
        "bias6": bias6,
        "slotq": slotq.reshape(4, 2048),
        "iota8": iota8,
        "ones1": np.ones((1, 128), np.float32),
        "edge": edge,
    }
    in_map.update(wt)
    return in_map, slotq


def kernel(x, y, u, q_w1, q_b1, q_w2, q_b2, k_w1, k_b1, k_w2, k_b2,
           v_w1, v_b1, v_w2, v_b2, proj_w, proj_b):
    x = np.asarray(x, np.float32)
    y = np.asarray(y, np.float32)
    u_in = np.asarray(u)
    uc = np.clip(u_in, 0, WW - 1).astype(np.int64)

    import ml_dtypes
    xr, yr = _round12(x), y    # y is cast to bf16 in _prep_core_a
    wsrc = {"wq1": q_w1, "wq2": q_w2, "wk1": k_w1, "wk2": k_w2,
            "wv1": v_w1, "wv2": v_w2}

    def _prep_w(n, w):
        wr = np.asarray(w, np.float32).transpose(1, 2, 3, 0).reshape(
            128, 9, 128)
        if W_DT[n] == "bf16":
            return np.ascontiguousarray(wr).astype(ml_dtypes.bfloat16)
        return _round12(wr)

    wt = {n: _prep_w(n, w) for n, w in wsrc.items()}
    bias6 = np.stack([
        np.asarray(q_b1, np.float32),
        np.asarray(q_b2, np.float32) * np.float32(SCALE),
        np.asarray(k_b1, np.float32), np.asarray(k_b2, np.float32),
        np.asarray(v_b1, np.float32), np.asarray(v_b2, np.float32),
    ], axis=1)                     # (128, 6)

    in_maps, slot_maps = [], []
    for c in range(NCORE):
        im, sq = _prep_core_a(xr, yr, uc, wt, bias6, c // 2, c % 2)
        in_maps.append(im)
        slot_maps.append(sq)

    global LAST_EXEC_NS, LAST_TRACES
    LAST_TRACES = []
    if TRACE:
        _ensure_ntff_hook()
    tkw = dict(trace=True, trace_cores=list(range(NCORE))) if TRACE else {}
    nc_a = _get_nc("a")
    res_a = run_bass_kernel_spmd(nc_a, in_maps, list(range(NCORE)), **tkw)

    a_full = np.zeros((BB, SS, 128), np.float32)
    for c in range(NCORE):
        flat = res_a.results[c]["a_out"].reshape(NSLOT, 128)
        sq = slot_maps[c]
        valid = sq < 1024
        a_full[c // 2][sq[valid].astype(np.int64)] = flat[valid]
    a_img = a_full.transpose(0, 2, 1).reshape(BB, 128, S, S)

    wpr = _round12(np.asarray(proj_w, np.float32)
                   .transpose(1, 2, 3, 0).reshape(128, 9, 128))
    bpr = np.asarray(proj_b, np.float32).reshape(128, 1)
    in_maps_b = []
    for c in range(NCORE):
        b, rh = c // 2, c % 2
        a_slab = np.zeros((128, 18, 34), np.float32)
        r0 = 16 * rh
        rlo, rhi = max(r0 - 1, 0), min(r0 + 17, S)
        a_slab[:, (rlo - (r0 - 1)):(rhi - (r0 - 1)), 1:33] = \
            _round12(a_img[b, :, rlo:rhi, :])
        in_maps_b.append({"a_slab": a_slab, "wp": wpr, "bp": bpr})

    nc_b = _get_nc("b")
    res_b = run_bass_kernel_spmd(nc_b, in_maps_b, list(range(NCORE)), **tkw)
    if TRACE:
        LAST_EXEC_NS = (res_a.exec_time_ns or 0) + (res_b.exec_time_ns or 0)
        LAST_TRACES = [res_a, res_b]

    z = np.zeros((BB, 128, S, S), np.float32)
    for c in range(NCORE):
        b, rh = c // 2, c % 2
        z[b, :, 16 * rh:16 * rh + 16, :] = \
            res_b.results[c]["z_out"].reshape(128, 16, 32)
    return z

